# revision 11
# baseline (speedup 1.0000x reference)
"""GPT-2 transformer block on 8 trn2 NeuronCores (Bass/Tile), v4.

Sharding: token-split. Core c = 4*b + j handles batch b, output tokens
[512j, 512j+512). Host reorders each core's sequence so the own tokens sit at
positions [0,512); K/V are computed for the full (reordered) sequence, Q and
everything downstream only for positions [0,512). Causal masking:
  - non-own keys are visible to all own queries or none (per-key), applied as
    an additive bias (-30) inside the exp activation (per-partition bias);
  - own keys (score tiles 0..3) get a triangular mask multiply.
v3: DMA-XBAR transposes, token-major V (bias folded into bo), bit-trick
    Newton reciprocal + head-pipelined softmax tail, paired exp activations,
    gpsimd mask multiplies, natural-layout MLP down-projection.
v4 (KERNEL_FP8=1): Q/K/V projections, the att@V matmuls (non-diagonal pairs)
    and the o-projection run in fp8e4 with DoubleRow (2 contraction rows per
    pass); per-tensor absmax scales travel in the `dq` input. MLP stays bf16
    (fp8 there breaks the 2e-2 error budget; measured offline).
KERNEL_SCHR=1: exp for the last two key-tile pairs is computed on the vector
    engine via the Schraudolph bit trick (bf16 out), rebalancing the
    scalar-engine exp bottleneck.
"""
import math
import os
import sys
import types

sys.path.insert(0, '/opt/trn_rl_repo')

import numpy as np
import ml_dtypes


def _install_ntff_shim():
    """concourse's trace path imports antenv.axon_hooks, which this image
    lacks; give it a functional stand-in so trace=True doesn't crash."""
    try:
        import antenv.axon_hooks  # noqa: F401
        return
    except ImportError:
        pass
    try:
        import antenv
    except ImportError:
        return
    mod = types.ModuleType("antenv.axon_hooks")
    mod._hook = None

    def set_axon_ntff_profile_hook(h):
        mod._hook = h

    def get_axon_ntff_profile_hook():
        return mod._hook

    mod.set_axon_ntff_profile_hook = set_axon_ntff_profile_hook
    mod.get_axon_ntff_profile_hook = get_axon_ntff_profile_hook
    sys.modules["antenv.axon_hooks"] = mod
    antenv.axon_hooks = mod
    try:
        from trn_agent_boot.trn_boot import _ntff_profile_via_ctypes
        hook = _ntff_profile_via_ctypes('/opt/axon/libaxon_pjrt.so')
        if hook is not None:
            set_axon_ntff_profile_hook(hook)
    except Exception:
        pass


_install_ntff_shim()

import concourse.bass as bass
import concourse.tile as tile
from concourse import mybir, bass_utils
from concourse.masks import make_identity

P = 128
B, S, E = 2, 2048, 2048
H, D, KH, G = 16, 128, 4, 4
F = 8192
OWN = 512                 # tokens owned per core
NE = E // P               # 16
NSK = S // P              # 16
NF = F // P               # 64
NMS = OWN // P            # 4
f32 = mybir.dt.float32
f32r = mybir.dt.float32r
i32 = mybir.dt.int32
i16 = mybir.dt.int16
bf16 = mybir.dt.bfloat16
fp8 = mybir.dt.float8e4
EXP_SCALE = 1.0 / math.sqrt(D)
NEGB = -30.0              # additive key bias for hidden keys (exp->~1e-13)
RCP_MAGIC = 0x7EF311C3    # fast-reciprocal seed; 1 Newton step -> ~0.26% max err
# Schraudolph exp, bf16-bits variant: bits16 = x*SCHR_A + (A*kb + SCHR_B)
SCHR_A0 = 12102203.161561485          # 2^23 / ln 2
SCHR_B0 = 1064986823.0 - 0.014 * 8388608.0   # centered (mean ratio ~1)
SX = 8.0                  # fp8 scale for LN1 output (|x1| < 30 whp)
SV = 16.0                 # fp8 scale for V values
SO = 16.0                 # fp8 scale for attention output
STRIP_TP = os.environ.get("KERNEL_STRIP_TP", "mm")  # 'dma' (XBAR) or 'mm' (PE)
MASK_ENG = os.environ.get("KERNEL_MASK_ENG", "gpsimd")  # 'gpsimd' or 'vector'
FP8 = os.environ.get("KERNEL_FP8", "1") == "1"
FP8_AV = FP8 and os.environ.get("KERNEL_FP8_AV", "1") == "1"
FP8_O = FP8 and os.environ.get("KERNEL_FP8_O", "1") == "1"
SCHR = os.environ.get("KERNEL_SCHR", "0") == "1"
DR = mybir.MatmulPerfMode.DoubleRow


def split_waits(nc, maxw=1):
    """This walrus build supports at most one sync-wait per instruction;
    hoist excess waits onto same-engine NoOps placed before the owner."""
    n = 0
    for fn in nc.m.functions:
        for blk in fn.blocks:
            new_insts = []
            for inst in blk.instructions:
                si = inst.sync_info
                if si is not None and si.on_wait and len(si.on_wait) > maxw:
                    waits = list(si.on_wait)
                    excess, keep = waits[:-maxw], waits[-maxw:]
                    for ci, w in enumerate(excess):
                        new_insts.append(mybir.InstNoOp(
                            name=f"{inst.name}-ws{ci}", engine=inst.engine,
                            sync_info=mybir.SyncInfo(on_wait=[w], on_update=[])))
                        n += 1
                    inst.sync_info = mybir.SyncInfo(
                        on_wait=keep, on_update=list(si.on_update or []))
                new_insts.append(inst)
            blk.instructions = new_insts
    return n


def _ln_tile(nc, pool, x_tile, eps_t, out_tile, post_scale=None):
    """out = (x - mean(x)) * rsqrt(var(x) + eps) [* post_scale] along free dim."""
    stats = pool.tile([P, E // 512, 6], f32, tag="ln_stats")
    for i in range(E // 512):
        nc.vector.bn_stats(out=stats[:, i, :], in_=x_tile[:, i * 512:(i + 1) * 512])
    mv = pool.tile([P, 2], f32, tag="ln_mv")
    nc.vector.bn_aggr(out=mv, in_=stats)
    rstd = pool.tile([P, 1], f32, tag="ln_rstd")
    nc.scalar.activation(out=rstd, in_=mv[:, 1:2],
                         func=mybir.ActivationFunctionType.Sqrt, bias=eps_t)
    nc.vector.reciprocal(out=rstd, in_=rstd)
    if post_scale is not None:
        nc.vector.tensor_scalar(out=rstd, in0=rstd, scalar1=post_scale,
                                scalar2=None, op0=mybir.AluOpType.mult)
    nc.vector.tensor_scalar(out=out_tile, in0=x_tile, scalar1=mv[:, 0:1],
                            scalar2=rstd, op0=mybir.AluOpType.subtract,
                            op1=mybir.AluOpType.mult)


def build():
    nc = bass.Bass("TRN2", target_bir_lowering=False, debug=False, num_devices=8)

    wdt = fp8 if FP8 else bf16
    xkv = nc.dram_tensor("xkv", [S, E], bf16, kind="ExternalInput").ap()
    xres = nc.dram_tensor("xres", [OWN, E], f32, kind="ExternalInput").ap()
    tri = nc.dram_tensor("tri", [P, NMS, OWN], bf16, kind="ExternalInput").ap()
    kbias = nc.dram_tensor("kbias", [P, NSK], f32, kind="ExternalInput").ap()
    wq_s = nc.dram_tensor("wq_s", [H, P, NE, P], wdt, kind="ExternalInput").ap()
    wk_s = nc.dram_tensor("wk_s", [KH, P, NE, P], wdt, kind="ExternalInput").ap()
    wv_n = nc.dram_tensor("wv_n", [NE, P, KH * D], wdt, kind="ExternalInput").ap()
    if FP8 and os.environ.get("KERNEL_FP8_O", "1") == "1":
        wo_r = nc.dram_tensor("wo_r", [H // 2, P, 2, E], fp8,
                              kind="ExternalInput").ap()
    else:
        wo_r = nc.dram_tensor("wo_r", [H, P, E], bf16, kind="ExternalInput").ap()
    wu_s = nc.dram_tensor("wu_s", [NF, P, NE, P], bf16, kind="ExternalInput").ap()
    wd_r = nc.dram_tensor("wd_r", [NF, P, E], bf16, kind="ExternalInput").ap()
    bq = nc.dram_tensor("bq", [P, H], f32, kind="ExternalInput").ap()
    bk = nc.dram_tensor("bk", [P, KH], f32, kind="ExternalInput").ap()
    bu = nc.dram_tensor("bu", [P, NF], f32, kind="ExternalInput").ap()
    bd_bc = nc.dram_tensor("bd_bc", [P, E], f32, kind="ExternalInput").ap()
    dq = nc.dram_tensor("dq", [P, 4], f32, kind="ExternalInput").ap()
    out_d = nc.dram_tensor("out", [OWN, E], f32, kind="ExternalOutput").ap()

    with tile.TileContext(nc) as tc:
        _build_body(nc, tc, locals())
    return nc


def _build_body(nc, tc, t_):
    xkv, xres_d, tri_d, kbias_d = t_["xkv"], t_["xres"], t_["tri"], t_["kbias"]
    wq_s, wk_s, wv_n, wo_r, wu_s, wd_r = (t_[k] for k in
                                          ("wq_s", "wk_s", "wv_n", "wo_r", "wu_s", "wd_r"))
    bq, bk, bu, bd_bc_d, dq_d = (t_[k] for k in ("bq", "bk", "bu", "bd_bc", "dq"))
    out_d = t_["out_d"]
    del t_
    Ident = mybir.ActivationFunctionType.Identity
    Exp = mybir.ActivationFunctionType.Exp
    Gelu = mybir.ActivationFunctionType.Gelu
    mult = mybir.AluOpType.mult
    add = mybir.AluOpType.add

    mask_eng = nc.gpsimd if MASK_ENG == 'gpsimd' else nc.vector

    with (
        tc.tile_pool(name="persist", bufs=1) as persist,
        tc.tile_pool(name="resid", bufs=1) as resid,
    ):
        eps_t = persist.tile([P, 1], f32)
        nc.vector.memset(eps_t, 1e-5)
        ones_col = persist.tile([P, 1], bf16)   # lhsT for denominator (K=P, M=1)
        nc.vector.memset(ones_col, 1.0)
        ones_row = persist.tile([1, P], f32r)   # lhsT for broadcast (K=1, M=P)
        nc.vector.memset(ones_row.bitcast(f32), 1.0)
        ones8 = persist.tile([P, 2, 1], fp8)    # DR lhsT for fp8 denominator
        nc.vector.memset(ones8, 1.0)
        if STRIP_TP == 'mm':
            ident = persist.tile([P, P], bf16)
            make_identity(nc, ident)
        bq_sb = persist.tile([P, H], f32)
        nc.sync.dma_start(out=bq_sb, in_=bq)
        bk_sb = persist.tile([P, KH], f32)
        nc.sync.dma_start(out=bk_sb, in_=bk)
        kb_sb = persist.tile([P, NSK], f32)     # per-key additive exp bias
        nc.sync.dma_start(out=kb_sb, in_=kbias_d)
        dq_sb = persist.tile([P, 4], f32)       # fp8 dequant scales
        nc.sync.dma_start(out=dq_sb, in_=dq_d)
        if SCHR:  # kb2 = kb*(A/2^16) + B/2^16 for the bf16-bits exp trick
            kb2_sb = persist.tile([P, NSK], f32)
            nc.vector.tensor_scalar(
                out=kb2_sb, in0=kb_sb, scalar1=SCHR_A0 / 65536.0,
                scalar2=SCHR_B0 / 65536.0, op0=mult, op1=add)
        tri_sb = persist.tile([P, NMS, OWN], bf16)  # triangular mask tiles

        with tc.tile_pool(name="qkv_keep", bufs=1) as qkv_keep:
            qT = [qkv_keep.tile([P, OWN], bf16, tag=f"qT{i}", name=f"qT{i}") for i in range(H)]
            kT = [qkv_keep.tile([P, S], bf16, tag=f"kT{i}", name=f"kT{i}") for i in range(KH)]
            vdt = fp8 if FP8_AV else bf16
            vtok = qkv_keep.tile([P, NSK, KH * D], vdt, name="vtok")
            xres = [resid.tile([P, E], f32, tag=f"xres{t}", name=f"xres{t}")
                    for t in range(NMS)]
            ln2_mv = [resid.tile([P, 2], f32, tag=f"l2m{t}", name=f"l2m{t}")
                      for t in range(NMS)]
            ln2_rstd = [resid.tile([P, 1], f32, tag=f"l2r{t}", name=f"l2r{t}")
                        for t in range(NMS)]

            # ---------------- Phase 1: LN1 + Q/K/V over reordered seq --------
            with (
                tc.tile_pool(name="p1", bufs=1) as p1,
                tc.tile_pool(name="ps1", bufs=1, space="PSUM") as ps1,
            ):
                # x for chunk 0 first: one wide DMA per chunk, 4 token-tiles
                x4s = []
                for c in range(S // OWN):
                    x4 = p1.tile([P, NMS, E], bf16, tag="x4", bufs=2, name=f"x4_{c}")
                    nc.sync.dma_start(
                        out=x4,
                        in_=xkv[c * OWN:(c + 1) * OWN, :].rearrange(
                            "(t p) e -> p t e", p=P))
                    x4s.append(x4)
                # wv tiles stay resident (reused by every chunk)
                wv_sb = p1.tile([P, NE, KH * D], fp8 if FP8 else bf16, name="wv_sb")
                nc.sync.dma_start(out=wv_sb, in_=wv_n.rearrange("e p m -> p e m"))

                def proj_mms(psum, wstrip, strip, n=NE):
                    if FP8:
                        for e in range(0, n, 2):
                            nc.tensor.matmul(psum, wstrip[:, e:e + 2, :],
                                             strip[:, e:e + 2, :],
                                             start=(e == 0), stop=(e == n - 2),
                                             perf_mode=DR)
                    else:
                        for e in range(n):
                            nc.tensor.matmul(psum, wstrip[:, e, :], strip[:, e, :],
                                             start=(e == 0), stop=(e == n - 1))

                for c in range(S // OWN):
                    x4 = x4s[c] if c < 2 else p1.tile(
                        [P, NMS, E], bf16, tag="x4", bufs=2, name=f"x4_{c}")
                    if c >= 2:
                        nc.sync.dma_start(
                            out=x4,
                            in_=xkv[c * OWN:(c + 1) * OWN, :].rearrange(
                                "(t p) e -> p t e", p=P))
                    strip_b = p1.tile([P, NE, OWN], bf16, tag="strip", bufs=2,
                                      name=f"strip{c}")
                    x1c = []
                    for t in range(NMS):
                        x1_t = p1.tile([P, E], bf16, tag=f"x1_{t}")
                        _ln_tile(nc, p1, x4[:, t, :], eps_t, x1_t,
                                 post_scale=SX if FP8 else None)
                        x1c.append(x1_t)
                        if STRIP_TP == 'dma':
                            for e in range(NE):
                                nc.sync.dma_start(
                                    out=strip_b[:, e, t * P:(t + 1) * P],
                                    in_=x1_t[:, e * P:(e + 1) * P],
                                    transpose=True)
                    if STRIP_TP == 'mm':
                        for e in range(NE):
                            tp4 = ps1.tile([P, NMS, P], f32, tag="tp4", bufs=2)
                            for t in range(NMS):
                                nc.tensor.matmul(tp4[:, t, :],
                                                 x1c[t][:, e * P:(e + 1) * P],
                                                 ident, start=True, stop=True)
                            nc.scalar.copy(strip_b[:, e, :],
                                           tp4.rearrange("p t q -> p (t q)"))
                    if FP8:
                        strip = p1.tile([P, NE, OWN], fp8, tag="strip8", bufs=2,
                                        name=f"strip8_{c}")
                        nc.vector.tensor_copy(
                            strip.rearrange("p e q -> p (e q)"),
                            strip_b.rearrange("p e q -> p (e q)"))
                    else:
                        strip = strip_b
                    # K projection for this chunk (d-major, like v2)
                    for m in range(KH):
                        wstrip = p1.tile([P, NE, P], wq_s.dtype, tag=f"w{m % 2}",
                                         bufs=2)
                        nc.sync.dma_start(out=wstrip, in_=wk_s[m])
                        pskv = ps1.tile([P, OWN], f32, tag=f"ps{m % 2}", bufs=2)
                        proj_mms(pskv, wstrip, strip)
                        nc.scalar.activation(
                            out=kT[m][:, c * OWN:(c + 1) * OWN],
                            in_=pskv, func=Ident, bias=bk_sb[:, m:m + 1],
                            scale=dq_sb[:, 1:2] if FP8 else 1.0)
                    # V projection, directly token-major (bias folded into bo)
                    for t in range(NMS):
                        psv = ps1.tile([P, KH * D], f32, tag=f"ps{t % 2}", bufs=2)
                        if FP8:
                            for e in range(0, NE, 2):
                                nc.tensor.matmul(
                                    psv, strip[:, e:e + 2, t * P:(t + 1) * P],
                                    wv_sb[:, e:e + 2, :],
                                    start=(e == 0), stop=(e == NE - 2),
                                    perf_mode=DR)
                            nc.scalar.activation(out=vtok[:, c * NMS + t, :],
                                                 in_=psv, func=Ident,
                                                 scale=dq_sb[:, 2:3])  # = SV/(SX*swv)
                        else:
                            for e in range(NE):
                                nc.tensor.matmul(
                                    psv, strip[:, e, t * P:(t + 1) * P],
                                    wv_sb[:, e, :],
                                    start=(e == 0), stop=(e == NE - 1))
                            nc.scalar.copy(vtok[:, c * NMS + t, :], psv)
                    if c == 0:
                        # Q projections for own tokens (positions [0, 512))
                        for m in range(H):
                            wstrip = p1.tile([P, NE, P], wq_s.dtype,
                                             tag=f"w{m % 2}", bufs=2)
                            nc.sync.dma_start(out=wstrip, in_=wq_s[m])
                            psq = ps1.tile([P, OWN], f32, tag=f"ps{m % 2}", bufs=2)
                            proj_mms(psq, wstrip, strip)
                            nc.scalar.activation(
                                out=qT[m], in_=psq, func=Ident,
                                bias=bq_sb[:, m:m + 1],
                                scale=dq_sb[:, 0:1] if FP8 else 1.0)

            # ---------------- Phase 2: attention -> oT -----------------------
            with tc.tile_pool(name="oT_keep", bufs=1) as oT_keep:
                odt = fp8 if FP8_O else bf16
                oT = oT_keep.tile([P, H, OWN], odt, name="oT")
                if FP8_O:
                    wo_pre = [oT_keep.tile([P, 2, E], fp8, tag=f"wopre{i}",
                                           name=f"wopre{i}") for i in range(2)]
                else:
                    wo_pre = [oT_keep.tile([P, E], bf16, tag=f"wopre{i}",
                                           name=f"wopre{i}") for i in range(2)]
                with (
                    tc.tile_pool(name="p2", bufs=1) as p2,
                    tc.tile_pool(name="ps2", bufs=1, space="PSUM") as ps2,
                ):
                    nc.sync.dma_start(out=tri_sb, in_=tri_d)
                    for i in range(2):
                        nc.sync.dma_start(out=wo_pre[i], in_=wo_r[i])
                    for t in range(NMS):
                        nc.sync.dma_start(out=xres[t], in_=xres_d[t * P:(t + 1) * P, :])

                    LOOK = 2
                    NPAIR = NSK // 2
                    prev = None   # deferred softmax tail state of head h-1

                    def nr_recip(st):
                        """rden = 1/ps_den via bit-trick + 1 Newton step."""
                        ps_den, rden = st["ps_den"], st["rden"]
                        nr_i = p2.tile([1, OWN], i32, tag="nr_i", bufs=2)
                        # walrus rejects mixed bitwise+arith in one tensor_scalar
                        nc.vector.tensor_scalar(
                            out=nr_i, in0=ps_den.bitcast(i32), scalar1=-1,
                            scalar2=None, op0=mybir.AluOpType.bitwise_xor)
                        nc.vector.tensor_scalar(
                            out=nr_i, in0=nr_i, scalar1=RCP_MAGIC + 1,
                            scalar2=None, op0=add)
                        nr_t = p2.tile([1, OWN], f32, tag="nr_t", bufs=2)
                        nc.vector.tensor_tensor(out=nr_t, in0=ps_den,
                                                in1=nr_i.bitcast(f32), op=mult)
                        nc.vector.tensor_scalar(out=nr_t, in0=nr_t, scalar1=-1.0,
                                                scalar2=2.0, op0=mult, op1=add)
                        with nc.allow_low_precision(reason="softmax recip"):
                            nc.vector.tensor_tensor(out=rden,
                                                    in0=nr_i.bitcast(f32),
                                                    in1=nr_t, op=mult)

                    def bc_mm(st):
                        ps_bc = ps2.tile([P, OWN], f32, tag="ps_bc", bufs=1)
                        nc.tensor.matmul(ps_bc, ones_row, st["rden"],
                                         start=True, stop=True)
                        st["ps_bc"] = ps_bc

                    def final_mult(st):
                        # DVE cannot read two PSUM operands; stage bc in SBUF
                        bc = p2.tile([P, OWN], f32, tag="bc", bufs=2)
                        nc.vector.tensor_copy(bc, st["ps_bc"])
                        nc.vector.tensor_tensor(out=oT[:, st["h"], :],
                                                in0=st["ps_o"],
                                                in1=bc, op=mult)

                    for h in range(H):
                        kv = h // G
                        ps_o = ps2.tile([P, OWN], f32, tag="ps_o", bufs=2)
                        acc = p2.tile([P, 2, OWN], bf16, tag="acc", bufs=2)
                        exrs = {}

                        def issue_pair(pr, kv=kv, h=h, exrs=exrs):
                            # pairs 0,1 carry the triangular mask (bf16 path);
                            # SCHR pairs 6,7 use the vector-engine exp trick
                            schr_pr = SCHR and pr >= NPAIR - 2
                            plain8 = FP8_AV and not schr_pr and pr >= 2
                            ps_s = ps2.tile([P, 2, OWN], f32, tag="ps_s", bufs=2)
                            exr2 = p2.tile([P, 2, OWN], fp8 if plain8 else bf16,
                                           tag="exr8" if plain8 else "exr",
                                           bufs=3)
                            for i in (0, 1):
                                sk = 2 * pr + i
                                nc.tensor.matmul(
                                    ps_s[:, i, :], kT[kv][:, sk * P:(sk + 1) * P],
                                    qT[h], start=True, stop=True)
                            if schr_pr:
                                nc.vector.tensor_scalar(
                                    out=exr2.rearrange("p t q -> p (t q)"
                                                       ).bitcast(i16),
                                    in0=ps_s.rearrange("p t q -> p (t q)"),
                                    scalar1=SCHR_A0 * EXP_SCALE / 65536.0,
                                    scalar2=kb2_sb[:, 2 * pr:2 * pr + 1],
                                    op0=mult, op1=add)
                            else:
                                nc.scalar.activation(
                                    out=exr2.rearrange("p t q -> p (t q)"),
                                    in_=ps_s.rearrange("p t q -> p (t q)"),
                                    func=Exp, scale=EXP_SCALE,
                                    bias=kb_sb[:, 2 * pr:2 * pr + 1])
                            if pr < 2:  # diagonal: triangular mask multiply
                                mask_eng.tensor_tensor(
                                    out=exr2.rearrange("p t q -> p (t q)"),
                                    in0=exr2.rearrange("p t q -> p (t q)"),
                                    in1=tri_sb[:, 2 * pr:2 * pr + 2, :].rearrange(
                                        "p t q -> p (t q)"), op=mult)
                            exrs[pr] = (exr2, plain8)

                        ps_den = ps2.tile([1, OWN], f32, tag="ps_den", bufs=1)
                        nbf = 0  # bf16 pairs seen (their den goes via acc)
                        nf8 = 0
                        den_started = [False]

                        for pr in range(LOOK):
                            issue_pair(pr)
                        if prev is not None:
                            nr_recip(prev)
                        for pr in range(NPAIR):
                            if pr + LOOK < NPAIR:
                                issue_pair(pr + LOOK)
                            exr2, plain8 = exrs[pr]
                            if plain8:
                                nc.tensor.matmul(
                                    ps_o,
                                    vtok[:, 2 * pr:2 * pr + 2,
                                         kv * D:(kv + 1) * D],
                                    exr2, start=(pr == 0), stop=(pr == NPAIR - 1),
                                    perf_mode=DR)
                                # denominator ride-along on PE (DR, M=1)
                                nc.tensor.matmul(
                                    ps_den, ones8, exr2,
                                    start=not den_started[0], stop=False,
                                    perf_mode=DR, skip_group_check=True)
                                den_started[0] = True
                            else:
                                for i in (0, 1):
                                    sk = 2 * pr + i
                                    nc.tensor.matmul(
                                        ps_o, vtok[:, sk, kv * D:(kv + 1) * D],
                                        exr2[:, i, :], start=(sk == 0),
                                        stop=(sk == NSK - 1))
                                with nc.allow_low_precision(reason="softmax den"):
                                    if nbf == 0:
                                        nc.vector.tensor_copy(
                                            acc.rearrange("p t q -> p (t q)"),
                                            exr2.rearrange("p t q -> p (t q)"))
                                    else:
                                        nc.vector.tensor_tensor(
                                            out=acc.rearrange("p t q -> p (t q)"),
                                            in0=acc.rearrange("p t q -> p (t q)"),
                                            in1=exr2.rearrange("p t q -> p (t q)"),
                                            op=add)
                                nbf += 1
                            if pr == 2 and prev is not None:
                                bc_mm(prev)
                            if pr == 4 and prev is not None:
                                final_mult(prev)
                        nc.tensor.matmul(ps_den, ones_col, acc[:, 0, :],
                                         start=not den_started[0], stop=False,
                                         skip_group_check=True)
                        nc.tensor.matmul(ps_den, ones_col, acc[:, 1, :],
                                         start=False, stop=True,
                                         skip_group_check=True)
                        rden = p2.tile([1, OWN], f32r, tag="rden", bufs=2)
                        prev = {"h": h, "ps_o": ps_o, "ps_den": ps_den, "rden": rden}
                    # flush the last head's tail
                    nr_recip(prev)
                    bc_mm(prev)
                    final_mult(prev)

                # ---------------- Phase 3: o-proj + residual -> xres ---------
                with (
                    tc.tile_pool(name="p3", bufs=1) as p3,
                    tc.tile_pool(name="ps3", bufs=1, space="PSUM") as ps3,
                ):
                    for mp in range(2):
                        pso = [ps3.tile([P, OWN], f32, tag=f"pso{i}", bufs=1,
                                        name=f"pso{i}") for i in range(8)]
                        nk = H // 2 if FP8_O else H
                        for k in range(nk):
                            if k < 2:
                                wtile = wo_pre[k]
                            else:
                                wtile = p3.tile(
                                    [P, 2, E] if FP8_O else [P, E],
                                    fp8 if FP8_O else bf16, tag="wo", bufs=3)
                                nc.sync.dma_start(out=wtile, in_=wo_r[k])
                            for ec in range(4):
                                for msi in range(2):
                                    ms = mp * 2 + msi
                                    if FP8_O:
                                        nc.tensor.matmul(
                                            pso[msi * 4 + ec],
                                            oT[:, 2 * k:2 * k + 2,
                                               ms * P:(ms + 1) * P],
                                            wtile[:, :, ec * OWN:(ec + 1) * OWN],
                                            start=(k == 0), stop=(k == nk - 1),
                                            perf_mode=DR)
                                    else:
                                        nc.tensor.matmul(
                                            pso[msi * 4 + ec],
                                            oT[:, k, ms * P:(ms + 1) * P],
                                            wtile[:, ec * OWN:(ec + 1) * OWN],
                                            start=(k == 0), stop=(k == nk - 1))
                        for msi in range(2):
                            ms = mp * 2 + msi
                            for ec in range(4):
                                lo = ec * OWN
                                src = pso[msi * 4 + ec]
                                if FP8_O:  # dequant on the (idle) scalar engine
                                    t8 = p3.tile([P, OWN], f32, tag="t8", bufs=3)
                                    nc.scalar.activation(
                                        out=t8, in_=src, func=Ident,
                                        scale=dq_sb[:, 3:4])
                                    src = t8
                                nc.vector.tensor_tensor(
                                    out=xres[ms][:, lo:lo + OWN],
                                    in0=src,
                                    in1=xres[ms][:, lo:lo + OWN], op=add)
                        # LN2 stats for this pair overlap the next pass
                        for msi in range(2):
                            ms = mp * 2 + msi
                            stats = p3.tile([P, E // 512, 6], f32, tag="ln_stats")
                            for i in range(E // 512):
                                nc.vector.bn_stats(
                                    out=stats[:, i, :],
                                    in_=xres[ms][:, i * 512:(i + 1) * 512])
                            nc.vector.bn_aggr(out=ln2_mv[ms], in_=stats)
                            nc.scalar.activation(
                                out=ln2_rstd[ms], in_=ln2_mv[ms][:, 1:2],
                                func=mybir.ActivationFunctionType.Sqrt, bias=eps_t)
                            nc.vector.reciprocal(out=ln2_rstd[ms], in_=ln2_rstd[ms])

        # ---------------- Phase 4: LN2 -> x2T strips; xres += bd ------------
        with tc.tile_pool(name="mlp_keep", bufs=1) as mlp_keep:
            x2T = mlp_keep.tile([P, NE, OWN], bf16, name="x2T")
            hT = [mlp_keep.tile([P, OWN], bf16, tag=f"hT{i}", name=f"hT{i}")
                  for i in range(NF)]
            wu_pre = [mlp_keep.tile([P, NE, P], bf16, tag=f"wupre{i}",
                                    name=f"wupre{i}") for i in range(2)]
            bd_sb = mlp_keep.tile([P, E], f32, name="bd_sb")
            nc.sync.dma_start(out=bd_sb, in_=bd_bc_d)

            with (
                tc.tile_pool(name="p4", bufs=1) as p4,
                tc.tile_pool(name="ps4", bufs=1, space="PSUM") as ps4,
            ):
                for i in range(2):
                    nc.sync.dma_start(out=wu_pre[i], in_=wu_s[i])
                for t in range(NMS):
                    x2_t = p4.tile([P, E], bf16, tag=f"x2_{t}", name=f"x2_{t}")
                    nc.vector.tensor_scalar(
                        out=x2_t, in0=xres[t], scalar1=ln2_mv[t][:, 0:1],
                        scalar2=ln2_rstd[t], op0=mybir.AluOpType.subtract,
                        op1=mybir.AluOpType.mult)
                    if STRIP_TP == 'dma':
                        for e in range(NE):
                            nc.sync.dma_start(
                                out=x2T[:, e, t * P:(t + 1) * P],
                                in_=x2_t[:, e * P:(e + 1) * P], transpose=True)
                    else:
                        for eg in range(4):
                            tp4 = ps4.tile([P, 4, P], f32, tag="tp4", bufs=2)
                            for i in range(4):
                                e = eg * 4 + i
                                nc.tensor.matmul(tp4[:, i, :],
                                                 x2_t[:, e * P:(e + 1) * P],
                                                 ident, start=True, stop=True)
                            for i in range(4):
                                e = eg * 4 + i
                                nc.scalar.copy(x2T[:, e, t * P:(t + 1) * P],
                                               tp4[:, i, :])
                    # bd folded into the residual stream after LN2 stats taken
                    nc.vector.tensor_tensor(out=xres[t], in0=xres[t],
                                            in1=bd_sb, op=add)

            # ---------------- Phase 5: MLP up (gelu) -> hT ------------------
            with (
                tc.tile_pool(name="p5", bufs=1) as p5,
                tc.tile_pool(name="ps5", bufs=1, space="PSUM") as ps5,
            ):
                bu_sb = p5.tile([P, NF], f32)
                nc.sync.dma_start(out=bu_sb, in_=bu)
                for f in range(NF):
                    if f < 2:
                        wstrip = wu_pre[f]
                    else:
                        wstrip = p5.tile([P, NE, P], bf16, tag=f"wu{f % 2}", bufs=3)
                        nc.sync.dma_start(out=wstrip, in_=wu_s[f])
                    psh = ps5.tile([P, OWN], f32, tag=f"psh{f % 2}", bufs=2)
                    for e in range(NE):
                        nc.tensor.matmul(psh, wstrip[:, e, :], x2T[:, e, :],
                                         start=(e == 0), stop=(e == NE - 1))
                    nc.scalar.activation(out=hT[f], in_=psh, func=Gelu,
                                         bias=bu_sb[:, f:f + 1])

            # ------------- Phase 6: MLP down (natural) + residual -----------
            with (
                tc.tile_pool(name="p6", bufs=1) as p6,
                tc.tile_pool(name="ps6", bufs=1, space="PSUM") as ps6,
            ):
                for eg in range(4):
                    psd = [ps6.tile([P, OWN], f32, tag=f"psd{eg % 2}_{i}", bufs=1,
                                    name=f"psd{eg % 2}_{i}") for i in range(4)]
                    for fi in range(NF):
                        wtile = p6.tile([P, OWN], bf16, tag=f"wd{fi % 2}", bufs=4)
                        nc.sync.dma_start(
                            out=wtile, in_=wd_r[fi][:, eg * OWN:(eg + 1) * OWN])
                        for qi in range(4):
                            nc.tensor.matmul(psd[qi],
                                             hT[fi][:, qi * P:(qi + 1) * P],
                                             wtile, start=(fi == 0),
                                             stop=(fi == NF - 1))
                    for qi in range(4):
                        lo = eg * OWN
                        nc.vector.tensor_tensor(
                            out=xres[qi][:, lo:lo + OWN], in0=psd[qi],
                            in1=xres[qi][:, lo:lo + OWN], op=add)
                        nc.sync.dma_start(
                            out=out_d[qi * P:(qi + 1) * P, lo:lo + OWN],
                            in_=xres[qi][:, lo:lo + OWN])


_NC_CACHE = None
LAST_RESULTS = None


def _get_nc():
    global _NC_CACHE
    if _NC_CACHE is None:
        nc = build()
        split_waits(nc)
        _NC_CACHE = nc
    return _NC_CACHE


def _q8(w, out_shape_fn=None):
    """absmax-quantize to fp8e4 (clip 240); returns (q, dequant_scale)."""
    amax = float(np.abs(w).max())
    s = 240.0 * 0.98 / max(amax, 1e-30)
    q = np.clip(w * s, -240, 240).astype(ml_dtypes.float8_e4m3)
    return q, 1.0 / s


def _prep_shared(ln1_g, ln1_b, wq, bq, wk, bk, wv, bv, wo, bo, ln2_g, ln2_b,
                 wu, bu, wd, bd):
    f = np.float64
    ln1_g, ln1_b = np.asarray(ln1_g, f), np.asarray(ln1_b, f)
    ln2_g, ln2_b = np.asarray(ln2_g, f), np.asarray(ln2_b, f)
    wq, wk, wv = np.asarray(wq, f), np.asarray(wk, f), np.asarray(wv, f)
    wo, wu, wd = np.asarray(wo, f), np.asarray(wu, f), np.asarray(wd, f)
    # fold LN gains into weights, LN biases into projection biases
    wq_f, bq_f = ln1_g[:, None] * wq, np.asarray(bq, f) + ln1_b @ wq
    wk_f, bk_f = ln1_g[:, None] * wk, np.asarray(bk, f) + ln1_b @ wk
    wv_f, bv_f = ln1_g[:, None] * wv, np.asarray(bv, f) + ln1_b @ wv
    wu_f, bu_f = ln2_g[:, None] * wu, np.asarray(bu, f) + ln2_b @ wu
    # V bias folds through attention (softmax rows sum to 1) into bo;
    # each kv head's bias is shared by its G query heads (GQA)
    bv_full = np.repeat(bv_f.reshape(KH, D), G, axis=0).reshape(H * D)
    bo_f = np.asarray(bo, f) + bv_full @ wo

    def strips(w, n, dt):  # [E, n*128] -> [n, 128(p), NE, 128(m)]
        return np.ascontiguousarray(
            w.reshape(NE, P, n, P).transpose(2, 1, 0, 3)).astype(dt)

    def rows(w, nr):   # [nr*128, E] -> [nr, 128, E]
        return np.ascontiguousarray(w.reshape(nr, P, E)).astype(ml_dtypes.bfloat16)

    tri = np.triu(np.ones((OWN, OWN), np.float32))  # [key, query]: k <= q
    tri = np.ascontiguousarray(
        tri.reshape(NMS, P, OWN).transpose(1, 0, 2)).astype(ml_dtypes.bfloat16)

    def ptile(v, n):  # [n*128] -> [128, n] (partition-major)
        return np.ascontiguousarray(
            np.asarray(v).reshape(n, P).T).astype(np.float32)

    shared = {
        "wu_s": strips(wu_f, NF, ml_dtypes.bfloat16), "wd_r": rows(wd, NF),
        "bq": ptile(bq_f, H), "bk": ptile(bk_f, KH),
        "bu": ptile(bu_f, NF),
        "bd_bc": np.ascontiguousarray(
            np.broadcast_to(np.asarray(bd, f)[None, :], (P, E))).astype(np.float32),
        "tri": tri,
    }
    if FP8:
        wq8, dqq = _q8(wq_f)
        wk8, dqk = _q8(wk_f)
        wv8, dqv = _q8(wv_f)
        wo8, dqo = _q8(wo)
        shared["wq_s"] = strips(wq8.astype(f), H, ml_dtypes.float8_e4m3)
        shared["wk_s"] = strips(wk8.astype(f), KH, ml_dtypes.float8_e4m3)
        shared["wv_n"] = np.ascontiguousarray(
            wv8.reshape(NE, P, KH * D))
        # wo pairs: [H//2, 128, 2, E]
        shared["wo_r"] = np.ascontiguousarray(
            wo8.reshape(H // 2, 2, P, E).transpose(0, 2, 1, 3))
        dqcols = np.array([dqq / SX, dqk / SX, dqv * SV / SX, dqo / (SO * 1.0)],
                          np.float32)
        shared["dq"] = np.ascontiguousarray(
            np.broadcast_to(dqcols[None, :], (P, 4))).astype(np.float32)
    else:
        shared["wq_s"] = strips(wq_f, H, ml_dtypes.bfloat16)
        shared["wk_s"] = strips(wk_f, KH, ml_dtypes.bfloat16)
        shared["wv_n"] = np.ascontiguousarray(
            wv_f.reshape(NE, P, KH * D)).astype(ml_dtypes.bfloat16)
        shared["wo_r"] = rows(wo, H)
        shared["dq"] = np.ones((P, 4), np.float32)
    return shared, bo_f


def kernel(x, ln1_g, ln1_b, wq, bq, wk, bk, wv, bv, wo, bo, ln2_g, ln2_b,
           wu, bu, wd, bd):
    x = np.asarray(x, np.float32)
    shared, bo_f = _prep_shared(ln1_g, ln1_b, wq, bq, wk, bk, wv, bv, wo, bo,
                                ln2_g, ln2_b, wu, bu, wd, bd)
    in_maps = []
    for core in range(8):
        b, j = divmod(core, 4)
        m = dict(shared)
        own = slice(OWN * j, OWN * (j + 1))
        # reorder: own tokens first, then the rest in natural order
        order = np.concatenate([np.arange(OWN * j, OWN * (j + 1)),
                                np.arange(0, OWN * j),
                                np.arange(OWN * (j + 1), S)])
        m["xkv"] = np.ascontiguousarray(x[b][order]).astype(ml_dtypes.bfloat16)
        m["xres"] = np.ascontiguousarray(x[b, own] + bo_f[None, :]).astype(np.float32)
        # per-key additive bias: 0 if key visible to all own queries (or own),
        # NEGB if hidden from all own queries
        kb = np.where(order < OWN * (j + 1), 0.0, NEGB).astype(np.float32)
        m["kbias"] = np.ascontiguousarray(kb.reshape(NSK, P).T).astype(np.float32)
        in_maps.append(m)

    nc = _get_nc()
    trace = bool(os.environ.get("KERNEL_TRACE"))
    res = bass_utils.run_bass_kernel_spmd(
        nc, in_maps, core_ids=list(range(8)), trace=trace)
    global LAST_RESULTS
    LAST_RESULTS = res
    out = np.empty((B, S, E), np.float32)
    for core in range(8):
        b, j = divmod(core, 4)
        out[b, OWN * j:OWN * (j + 1)] = res.results[core]["out"]
    return out


# revision 17
# speedup vs baseline: 1.2533x; 1.2533x over previous
"""GPT-2 transformer block on 8 trn2 NeuronCores (Bass/Tile), v4.

Sharding: token-split. Core c = 4*b + j handles batch b, output tokens
[512j, 512j+512). Host reorders each core's sequence so the own tokens sit at
positions [0,512); K/V are computed for the full (reordered) sequence, Q and
everything downstream only for positions [0,512). Causal masking:
  - non-own keys are visible to all own queries or none (per-key), applied as
    an additive bias (-30) inside the exp activation (per-partition bias);
  - own keys (score tiles 0..3) get a triangular mask multiply.
v3: DMA-XBAR transposes, token-major V (bias folded into bo), bit-trick
    Newton reciprocal + head-pipelined softmax tail, paired exp activations,
    gpsimd mask multiplies, natural-layout MLP down-projection.
v4 (KERNEL_FP8=1): Q/K/V projections, the att@V matmuls (non-diagonal pairs)
    and the o-projection run in fp8e4 with DoubleRow (2 contraction rows per
    pass); per-tensor absmax scales travel in the `dq` input. MLP stays bf16
    (fp8 there breaks the 2e-2 error budget; measured offline).
KERNEL_SCHR=1: exp for the last two key-tile pairs is computed on the vector
    engine via the Schraudolph bit trick (bf16 out), rebalancing the
    scalar-engine exp bottleneck.
"""
import math
import os
import sys
import types

sys.path.insert(0, '/opt/trn_rl_repo')

import numpy as np
import ml_dtypes


def _install_ntff_shim():
    """concourse's trace path imports antenv.axon_hooks, which this image
    lacks; give it a functional stand-in so trace=True doesn't crash."""
    try:
        import antenv.axon_hooks  # noqa: F401
        return
    except ImportError:
        pass
    try:
        import antenv
    except ImportError:
        return
    mod = types.ModuleType("antenv.axon_hooks")
    mod._hook = None

    def set_axon_ntff_profile_hook(h):
        mod._hook = h

    def get_axon_ntff_profile_hook():
        return mod._hook

    mod.set_axon_ntff_profile_hook = set_axon_ntff_profile_hook
    mod.get_axon_ntff_profile_hook = get_axon_ntff_profile_hook
    sys.modules["antenv.axon_hooks"] = mod
    antenv.axon_hooks = mod
    try:
        from trn_agent_boot.trn_boot import _ntff_profile_via_ctypes
        hook = _ntff_profile_via_ctypes('/opt/axon/libaxon_pjrt.so')
        if hook is not None:
            set_axon_ntff_profile_hook(hook)
    except Exception:
        pass


_install_ntff_shim()

import concourse.bass as bass
import concourse.tile as tile
from concourse import mybir, bass_utils
from concourse.masks import make_identity

P = 128
B, S, E = 2, 2048, 2048
H, D, KH, G = 16, 128, 4, 4
F = 8192
OWN = 512                 # tokens owned per core
NE = E // P               # 16
NSK = S // P              # 16
NF = F // P               # 64
NMS = OWN // P            # 4
f32 = mybir.dt.float32
f32r = mybir.dt.float32r
i32 = mybir.dt.int32
i16 = mybir.dt.int16
bf16 = mybir.dt.bfloat16
fp8 = mybir.dt.float8e4
EXP_SCALE = 1.0 / math.sqrt(D)
NEGB = -30.0              # additive key bias for hidden keys (exp->~1e-13)
EXPC = 3.5                # subtract from every logit: keeps exp < 240 so the
                          # fp8e4 store can't hit TRN's (240,448]->NaN band;
                          # cancels exactly in softmax (den shifts too)
RCP_MAGIC = 0x7EF311C3    # fast-reciprocal seed; 1 Newton step -> ~0.26% max err
# Schraudolph exp, bf16-bits variant: bits16 = x*SCHR_A + (A*kb + SCHR_B)
SCHR_A0 = 12102203.161561485          # 2^23 / ln 2
SCHR_B0 = 1064986823.0 - 0.014 * 8388608.0   # centered (mean ratio ~1)
SX = 8.0                  # fp8 scale for LN1 output (|x1| < 30 whp)
SV = 16.0                 # fp8 scale for V values
SO = 16.0                 # fp8 scale for attention output
STRIP_TP = os.environ.get("KERNEL_STRIP_TP", "dram")  # 'dma' (XBAR) or 'mm' (PE)
MASK_ENG = os.environ.get("KERNEL_MASK_ENG", "gpsimd")  # 'gpsimd' or 'vector'
FP8 = os.environ.get("KERNEL_FP8", "1") == "1"
FP8_AV = FP8 and os.environ.get("KERNEL_FP8_AV", "1") == "1"
FP8_O = FP8 and os.environ.get("KERNEL_FP8_O", "1") == "1"
SCHR = os.environ.get("KERNEL_SCHR", "1") == "1"
DR = mybir.MatmulPerfMode.DoubleRow


def split_waits(nc, maxw=1):
    """This walrus build supports at most one sync-wait per instruction;
    hoist excess waits onto same-engine NoOps placed before the owner."""
    n = 0
    for fn in nc.m.functions:
        for blk in fn.blocks:
            new_insts = []
            for inst in blk.instructions:
                si = inst.sync_info
                if si is not None and si.on_wait and len(si.on_wait) > maxw:
                    waits = list(si.on_wait)
                    excess, keep = waits[:-maxw], waits[-maxw:]
                    for ci, w in enumerate(excess):
                        new_insts.append(mybir.InstNoOp(
                            name=f"{inst.name}-ws{ci}", engine=inst.engine,
                            sync_info=mybir.SyncInfo(on_wait=[w], on_update=[])))
                        n += 1
                    inst.sync_info = mybir.SyncInfo(
                        on_wait=keep, on_update=list(si.on_update or []))
                new_insts.append(inst)
            blk.instructions = new_insts
    return n


def _ln_tile(nc, pool, x_tile, eps_t, out_tile, post_scale=None):
    """out = (x - mean(x)) * rsqrt(var(x) + eps) [* post_scale] along free dim."""
    stats = pool.tile([P, E // 512, 6], f32, tag="ln_stats")
    for i in range(E // 512):
        nc.vector.bn_stats(out=stats[:, i, :], in_=x_tile[:, i * 512:(i + 1) * 512])
    mv = pool.tile([P, 2], f32, tag="ln_mv")
    nc.vector.bn_aggr(out=mv, in_=stats)
    rstd = pool.tile([P, 1], f32, tag="ln_rstd")
    nc.scalar.activation(out=rstd, in_=mv[:, 1:2],
                         func=mybir.ActivationFunctionType.Sqrt, bias=eps_t)
    nc.vector.reciprocal(out=rstd, in_=rstd)
    if post_scale is not None:
        nc.vector.tensor_scalar(out=rstd, in0=rstd, scalar1=post_scale,
                                scalar2=None, op0=mybir.AluOpType.mult)
    nc.vector.tensor_scalar(out=out_tile, in0=x_tile, scalar1=mv[:, 0:1],
                            scalar2=rstd, op0=mybir.AluOpType.subtract,
                            op1=mybir.AluOpType.mult)


def build():
    nc = bass.Bass("TRN2", target_bir_lowering=False, debug=False, num_devices=8)

    wdt = fp8 if FP8 else bf16
    xkv = nc.dram_tensor("xkv", [S, E], bf16, kind="ExternalInput").ap()
    xres = nc.dram_tensor("xres", [OWN, E], f32, kind="ExternalInput").ap()
    tri = nc.dram_tensor("tri", [P, NMS, OWN], bf16, kind="ExternalInput").ap()
    kbias = nc.dram_tensor("kbias", [P, NSK], f32, kind="ExternalInput").ap()
    wq_s = nc.dram_tensor("wq_s", [H, P, NE, P], wdt, kind="ExternalInput").ap()
    wk_s = nc.dram_tensor("wk_s", [KH, P, NE, P], wdt, kind="ExternalInput").ap()
    wv_n = nc.dram_tensor("wv_n", [NE, P, KH * D], wdt, kind="ExternalInput").ap()
    if FP8 and os.environ.get("KERNEL_FP8_O", "1") == "1":
        wo_r = nc.dram_tensor("wo_r", [H // 2, P, 2, E], fp8,
                              kind="ExternalInput").ap()
    else:
        wo_r = nc.dram_tensor("wo_r", [H, P, E], bf16, kind="ExternalInput").ap()
    wu_s = nc.dram_tensor("wu_s", [NF, P, NE, P], bf16, kind="ExternalInput").ap()
    wd_r = nc.dram_tensor("wd_r", [NF, P, E], bf16, kind="ExternalInput").ap()
    bq = nc.dram_tensor("bq", [P, H], f32, kind="ExternalInput").ap()
    bk = nc.dram_tensor("bk", [P, KH], f32, kind="ExternalInput").ap()
    bu = nc.dram_tensor("bu", [P, NF], f32, kind="ExternalInput").ap()
    bd_bc = nc.dram_tensor("bd_bc", [P, E], f32, kind="ExternalInput").ap()
    dq = nc.dram_tensor("dq", [P, 4], f32, kind="ExternalInput").ap()
    x1_scr = nc.dram_tensor("x1_scr", [2, OWN, E], bf16, kind="Internal").ap()
    out_d = nc.dram_tensor("out", [OWN, E], f32, kind="ExternalOutput").ap()

    with tile.TileContext(nc) as tc:
        _build_body(nc, tc, locals())
    return nc


def _build_body(nc, tc, t_):
    xkv, xres_d, tri_d, kbias_d = t_["xkv"], t_["xres"], t_["tri"], t_["kbias"]
    x1_scr = t_["x1_scr"]
    wq_s, wk_s, wv_n, wo_r, wu_s, wd_r = (t_[k] for k in
                                          ("wq_s", "wk_s", "wv_n", "wo_r", "wu_s", "wd_r"))
    bq, bk, bu, bd_bc_d, dq_d = (t_[k] for k in ("bq", "bk", "bu", "bd_bc", "dq"))
    out_d = t_["out_d"]
    del t_
    Ident = mybir.ActivationFunctionType.Identity
    Exp = mybir.ActivationFunctionType.Exp
    Gelu = mybir.ActivationFunctionType.Gelu
    mult = mybir.AluOpType.mult
    add = mybir.AluOpType.add

    mask_eng = nc.gpsimd if MASK_ENG == 'gpsimd' else nc.vector

    with (
        tc.tile_pool(name="persist", bufs=1) as persist,
        tc.tile_pool(name="resid", bufs=1) as resid,
    ):
        eps_t = persist.tile([P, 1], f32)
        nc.vector.memset(eps_t, 1e-5)
        ones_col = persist.tile([P, 1], bf16)   # lhsT for denominator (K=P, M=1)
        nc.vector.memset(ones_col, 1.0)
        sv_eff = SV if FP8_AV else 1.0
        ones_row = persist.tile([1, P], f32r)   # lhsT for broadcast (K=1, M=P)
        nc.vector.memset(ones_row.bitcast(f32), 1.0 if FP8_O else 1.0 / sv_eff)
        # DR lhsT for fp8 denominator; ko-step must be 16B-aligned, so pad
        ones8_t = persist.tile([P, 2, 16], fp8)
        nc.vector.memset(ones8_t, 1.0)
        ones8 = ones8_t[:, :, 0:1]
        if STRIP_TP == 'mm':
            ident = persist.tile([P, P], bf16)
            make_identity(nc, ident)
        bq_sb = persist.tile([P, H], f32)
        nc.sync.dma_start(out=bq_sb, in_=bq)
        bk_sb = persist.tile([P, KH], f32)
        nc.sync.dma_start(out=bk_sb, in_=bk)
        kb_sb = persist.tile([P, NSK], f32)     # per-key additive exp bias
        nc.sync.dma_start(out=kb_sb, in_=kbias_d)
        dq_sb = persist.tile([P, 4], f32)       # fp8 dequant scales
        nc.sync.dma_start(out=dq_sb, in_=dq_d)
        if SCHR:  # kb2 = kb*(A/2^16) + B/2^16 for the bf16-bits exp trick
            kb2_sb = persist.tile([P, NSK], f32)
            nc.vector.tensor_scalar(
                out=kb2_sb, in0=kb_sb, scalar1=SCHR_A0 / 65536.0,
                scalar2=SCHR_B0 / 65536.0, op0=mult, op1=add)
        tri_sb = persist.tile([P, NMS, OWN], bf16)  # triangular mask tiles

        with tc.tile_pool(name="qkv_keep", bufs=1) as qkv_keep:
            qT = [qkv_keep.tile([P, OWN], bf16, tag=f"qT{i}", name=f"qT{i}") for i in range(H)]
            kT = [qkv_keep.tile([P, S], bf16, tag=f"kT{i}", name=f"kT{i}") for i in range(KH)]
            vdt = fp8 if FP8_AV else bf16
            vtok = qkv_keep.tile([P, NSK, KH * D], vdt, name="vtok")
            xres = [resid.tile([P, E], f32, tag=f"xres{t}", name=f"xres{t}")
                    for t in range(NMS)]
            ln2_mv = [resid.tile([P, 2], f32, tag=f"l2m{t}", name=f"l2m{t}")
                      for t in range(NMS)]
            ln2_rstd = [resid.tile([P, 1], f32, tag=f"l2r{t}", name=f"l2r{t}")
                        for t in range(NMS)]

            # ---------------- Phase 1: LN1 + Q/K/V over reordered seq --------
            with (
                tc.tile_pool(name="p1", bufs=1) as p1,
                tc.tile_pool(name="ps1", bufs=1, space="PSUM") as ps1,
            ):
                # x for chunk 0 first: one wide DMA per chunk, 4 token-tiles
                x4s = []
                for c in range(S // OWN):
                    x4 = p1.tile([P, NMS, E], bf16, tag="x4", bufs=2, name=f"x4_{c}")
                    nc.sync.dma_start(
                        out=x4,
                        in_=xkv[c * OWN:(c + 1) * OWN, :].rearrange(
                            "(t p) e -> p t e", p=P))
                    x4s.append(x4)
                # wv tiles stay resident (reused by every chunk)
                wv_sb = p1.tile([P, NE, KH * D], fp8 if FP8 else bf16, name="wv_sb")
                nc.sync.dma_start(out=wv_sb, in_=wv_n.rearrange("e p m -> p e m"))

                def proj_mms(psum, wstrip, strip, n=NE):
                    if FP8:
                        for e in range(0, n, 2):
                            nc.tensor.matmul(psum, wstrip[:, e:e + 2, :],
                                             strip[:, e:e + 2, :],
                                             start=(e == 0), stop=(e == n - 2),
                                             perf_mode=DR)
                    else:
                        for e in range(n):
                            nc.tensor.matmul(psum, wstrip[:, e, :], strip[:, e, :],
                                             start=(e == 0), stop=(e == n - 1))

                for c in range(S // OWN):
                    x4 = x4s[c] if c < 2 else p1.tile(
                        [P, NMS, E], bf16, tag="x4", bufs=2, name=f"x4_{c}")
                    if c >= 2:
                        nc.sync.dma_start(
                            out=x4,
                            in_=xkv[c * OWN:(c + 1) * OWN, :].rearrange(
                                "(t p) e -> p t e", p=P))
                    strip_b = p1.tile([P, NE, OWN], bf16, tag="strip", bufs=2,
                                      name=f"strip{c}")
                    x1c = []
                    for t in range(NMS):
                        x1_t = p1.tile([P, E], bf16, tag=f"x1_{t}")
                        _ln_tile(nc, p1, x4[:, t, :], eps_t, x1_t,
                                 post_scale=SX if FP8 else None)
                        x1c.append(x1_t)
                        if STRIP_TP == 'dma':
                            for e in range(NE):
                                nc.sync.dma_start(
                                    out=strip_b[:, e, t * P:(t + 1) * P],
                                    in_=x1_t[:, e * P:(e + 1) * P],
                                    transpose=True)
                        elif STRIP_TP == 'dram':
                            nc.sync.dma_start(
                                out=x1_scr[c % 2, t * P:(t + 1) * P, :],
                                in_=x1_t)
                    if STRIP_TP == 'dram':
                        for e in range(NE):
                            nc.sync.dma_start(
                                out=strip_b[:, e, :],
                                in_=x1_scr[c % 2, :, e * P:(e + 1) * P],
                                transpose=True)
                    if STRIP_TP == 'mm':
                        for e in range(NE):
                            tp4 = ps1.tile([P, NMS, P], f32, tag="tp4", bufs=2)
                            for t in range(NMS):
                                nc.tensor.matmul(tp4[:, t, :],
                                                 x1c[t][:, e * P:(e + 1) * P],
                                                 ident, start=True, stop=True)
                            nc.scalar.copy(strip_b[:, e, :],
                                           tp4.rearrange("p t q -> p (t q)"))
                    if FP8:
                        strip = p1.tile([P, NE, OWN], fp8, tag="strip8", bufs=2,
                                        name=f"strip8_{c}")
                        nc.vector.tensor_copy(
                            strip.rearrange("p e q -> p (e q)"),
                            strip_b.rearrange("p e q -> p (e q)"))
                    else:
                        strip = strip_b
                    # K projection for this chunk (d-major, like v2)
                    for m in range(KH):
                        wstrip = p1.tile([P, NE, P], wq_s.dtype, tag=f"w{m % 2}",
                                         bufs=2)
                        nc.sync.dma_start(out=wstrip, in_=wk_s[m])
                        pskv = ps1.tile([P, OWN], f32, tag=f"ps{m % 2}", bufs=2)
                        proj_mms(pskv, wstrip, strip)
                        nc.scalar.activation(
                            out=kT[m][:, c * OWN:(c + 1) * OWN],
                            in_=pskv, func=Ident, bias=bk_sb[:, m:m + 1],
                            scale=dq_sb[:, 1:2] if FP8 else 1.0)
                    # V projection, directly token-major (bias folded into bo)
                    for t in range(NMS):
                        psv = ps1.tile([P, KH * D], f32, tag=f"ps{t % 2}", bufs=2)
                        if FP8:
                            for e in range(0, NE, 2):
                                nc.tensor.matmul(
                                    psv, strip[:, e:e + 2, t * P:(t + 1) * P],
                                    wv_sb[:, e:e + 2, :],
                                    start=(e == 0), stop=(e == NE - 2),
                                    perf_mode=DR)
                            nc.scalar.activation(out=vtok[:, c * NMS + t, :],
                                                 in_=psv, func=Ident,
                                                 scale=dq_sb[:, 2:3])  # = SV/(SX*swv)
                        else:
                            for e in range(NE):
                                nc.tensor.matmul(
                                    psv, strip[:, e, t * P:(t + 1) * P],
                                    wv_sb[:, e, :],
                                    start=(e == 0), stop=(e == NE - 1))
                            nc.scalar.copy(vtok[:, c * NMS + t, :], psv)
                    if c == 0:
                        # Q projections for own tokens (positions [0, 512))
                        for m in range(H):
                            wstrip = p1.tile([P, NE, P], wq_s.dtype,
                                             tag=f"w{m % 2}", bufs=2)
                            nc.sync.dma_start(out=wstrip, in_=wq_s[m])
                            psq = ps1.tile([P, OWN], f32, tag=f"ps{m % 2}", bufs=2)
                            proj_mms(psq, wstrip, strip)
                            nc.scalar.activation(
                                out=qT[m], in_=psq, func=Ident,
                                bias=bq_sb[:, m:m + 1],
                                scale=dq_sb[:, 0:1] if FP8 else 1.0)

            # ---------------- Phase 2: attention -> oT -----------------------
            with tc.tile_pool(name="oT_keep", bufs=1) as oT_keep:
                odt = fp8 if FP8_O else bf16
                oT = oT_keep.tile([P, H, OWN], odt, name="oT")
                if FP8_O:
                    wo_pre = [oT_keep.tile([P, 2, E], fp8, tag=f"wopre{i}",
                                           name=f"wopre{i}") for i in range(2)]
                else:
                    wo_pre = [oT_keep.tile([P, E], bf16, tag=f"wopre{i}",
                                           name=f"wopre{i}") for i in range(2)]
                with (
                    tc.tile_pool(name="p2", bufs=1) as p2,
                    tc.tile_pool(name="ps2", bufs=1, space="PSUM") as ps2,
                ):
                    nc.sync.dma_start(out=tri_sb, in_=tri_d)
                    for i in range(2):
                        nc.sync.dma_start(out=wo_pre[i], in_=wo_r[i])
                    for t in range(NMS):
                        nc.sync.dma_start(out=xres[t], in_=xres_d[t * P:(t + 1) * P, :])

                    LOOK = 2
                    NPAIR = NSK // 2
                    prev = None   # deferred softmax tail state of head h-1

                    def nr_recip(st):
                        with nc.allow_low_precision(reason="softmax recip"):
                            nc.vector.reciprocal(out=st["rden"], in_=st["ps_den"])

                    def bc_mm(st):
                        ps_bc = ps2.tile([P, OWN], f32, tag="ps_bc", bufs=1)
                        nc.tensor.matmul(ps_bc, ones_row, st["rden"],
                                         start=True, stop=True)
                        st["ps_bc"] = ps_bc

                    def final_mult(st):
                        # DVE cannot read two PSUM operands; stage bc in SBUF
                        bc = p2.tile([P, OWN], f32, tag="bc", bufs=2)
                        nc.vector.tensor_copy(bc, st["ps_bc"])
                        nc.vector.tensor_tensor(out=oT[:, st["h"], :],
                                                in0=st["ps_o"],
                                                in1=bc, op=mult)

                    for h in range(H):
                        kv = h // G
                        ps_o = ps2.tile([P, OWN], f32, tag="ps_o", bufs=2)
                        acc = p2.tile([P, 2, OWN], bf16, tag="acc", bufs=2)
                        exrs = {}

                        def issue_pair(pr, kv=kv, h=h, exrs=exrs):
                            # pairs 0,1 carry the triangular mask (bf16 path);
                            # SCHR pairs 6,7 use the vector-engine exp trick
                            schr_pr = SCHR and pr >= NPAIR - 2
                            plain8 = FP8_AV and not schr_pr and pr >= 2
                            ps_s = ps2.tile([P, 2, OWN], f32, tag="ps_s", bufs=2)
                            exr2 = p2.tile([P, 2, OWN], fp8 if plain8 else bf16,
                                           tag="exr8" if plain8 else "exr",
                                           bufs=3)
                            for i in (0, 1):
                                sk = 2 * pr + i
                                nc.tensor.matmul(
                                    ps_s[:, i, :], kT[kv][:, sk * P:(sk + 1) * P],
                                    qT[h], start=True, stop=True)
                            if schr_pr:
                                nc.vector.tensor_scalar(
                                    out=exr2.rearrange("p t q -> p (t q)"
                                                       ).bitcast(i16),
                                    in0=ps_s.rearrange("p t q -> p (t q)"),
                                    scalar1=SCHR_A0 * EXP_SCALE / 65536.0,
                                    scalar2=kb2_sb[:, 2 * pr:2 * pr + 1],
                                    op0=mult, op1=add)
                            else:
                                nc.scalar.activation(
                                    out=exr2.rearrange("p t q -> p (t q)"),
                                    in_=ps_s.rearrange("p t q -> p (t q)"),
                                    func=Exp, scale=EXP_SCALE,
                                    bias=kb_sb[:, 2 * pr:2 * pr + 1])
                            if pr < 2:  # diagonal: triangular mask multiply
                                mask_eng.tensor_tensor(
                                    out=exr2.rearrange("p t q -> p (t q)"),
                                    in0=exr2.rearrange("p t q -> p (t q)"),
                                    in1=tri_sb[:, 2 * pr:2 * pr + 2, :].rearrange(
                                        "p t q -> p (t q)"), op=mult)
                            exrs[pr] = (exr2, plain8)

                        ps_den = ps2.tile([1, OWN], f32, tag="ps_den", bufs=1)
                        nbf = 0  # bf16 pairs seen (their den goes via acc)
                        nf8 = 0
                        den_started = [False]

                        for pr in range(LOOK):
                            issue_pair(pr)
                        if prev is not None:
                            nr_recip(prev)
                        for pr in range(NPAIR):
                            if pr + LOOK < NPAIR:
                                issue_pair(pr + LOOK)
                            exr2, plain8 = exrs[pr]
                            if plain8:
                                nc.tensor.matmul(
                                    ps_o,
                                    vtok[:, 2 * pr:2 * pr + 2,
                                         kv * D:(kv + 1) * D],
                                    exr2, start=(pr == 0), stop=(pr == NPAIR - 1),
                                    perf_mode=DR)
                                # denominator ride-along on PE (DR, M=1)
                                nc.tensor.matmul(
                                    ps_den, ones8, exr2,
                                    start=not den_started[0], stop=False,
                                    perf_mode=DR, skip_group_check=True)
                                den_started[0] = True
                            else:
                                for i in (0, 1):
                                    sk = 2 * pr + i
                                    nc.tensor.matmul(
                                        ps_o, vtok[:, sk, kv * D:(kv + 1) * D],
                                        exr2[:, i, :], start=(sk == 0),
                                        stop=(sk == NSK - 1))
                                with nc.allow_low_precision(reason="softmax den"):
                                    if nbf == 0:
                                        mask_eng.tensor_copy(
                                            acc.rearrange("p t q -> p (t q)"),
                                            exr2.rearrange("p t q -> p (t q)"))
                                    else:
                                        mask_eng.tensor_tensor(
                                            out=acc.rearrange("p t q -> p (t q)"),
                                            in0=acc.rearrange("p t q -> p (t q)"),
                                            in1=exr2.rearrange("p t q -> p (t q)"),
                                            op=add)
                                nbf += 1
                            if pr == 2 and prev is not None:
                                bc_mm(prev)
                            if pr == 4 and prev is not None:
                                final_mult(prev)
                        nc.tensor.matmul(ps_den, ones_col, acc[:, 0, :],
                                         start=not den_started[0], stop=False,
                                         skip_group_check=True)
                        nc.tensor.matmul(ps_den, ones_col, acc[:, 1, :],
                                         start=False, stop=True,
                                         skip_group_check=True)
                        rden = p2.tile([1, OWN], f32r, tag="rden", bufs=2)
                        prev = {"h": h, "ps_o": ps_o, "ps_den": ps_den, "rden": rden}
                    # flush the last head's tail
                    nr_recip(prev)
                    bc_mm(prev)
                    final_mult(prev)

                # ---------------- Phase 3: o-proj + residual -> xres ---------
                with (
                    tc.tile_pool(name="p3", bufs=1) as p3,
                    tc.tile_pool(name="ps3", bufs=1, space="PSUM") as ps3,
                ):
                    for mp in range(2):
                        pso = [ps3.tile([P, OWN], f32, tag=f"pso{i}", bufs=1,
                                        name=f"pso{i}") for i in range(8)]
                        nk = H // 2 if FP8_O else H
                        for k in range(nk):
                            if k < 2:
                                wtile = wo_pre[k]
                            else:
                                wtile = p3.tile(
                                    [P, 2, E] if FP8_O else [P, E],
                                    fp8 if FP8_O else bf16, tag="wo", bufs=3)
                                nc.sync.dma_start(out=wtile, in_=wo_r[k])
                            for ec in range(4):
                                for msi in range(2):
                                    ms = mp * 2 + msi
                                    if FP8_O:
                                        nc.tensor.matmul(
                                            pso[msi * 4 + ec],
                                            oT[:, 2 * k:2 * k + 2,
                                               ms * P:(ms + 1) * P],
                                            wtile[:, :, ec * OWN:(ec + 1) * OWN],
                                            start=(k == 0), stop=(k == nk - 1),
                                            perf_mode=DR)
                                    else:
                                        nc.tensor.matmul(
                                            pso[msi * 4 + ec],
                                            oT[:, k, ms * P:(ms + 1) * P],
                                            wtile[:, ec * OWN:(ec + 1) * OWN],
                                            start=(k == 0), stop=(k == nk - 1))
                        for msi in range(2):
                            ms = mp * 2 + msi
                            for ec in range(4):
                                lo = ec * OWN
                                src = pso[msi * 4 + ec]
                                if FP8_O:  # dequant on the (idle) scalar engine
                                    t8 = p3.tile([P, OWN], f32, tag="t8", bufs=3)
                                    nc.scalar.activation(
                                        out=t8, in_=src, func=Ident,
                                        scale=dq_sb[:, 3:4])
                                    src = t8
                                nc.vector.tensor_tensor(
                                    out=xres[ms][:, lo:lo + OWN],
                                    in0=src,
                                    in1=xres[ms][:, lo:lo + OWN], op=add)
                        # LN2 stats for this pair overlap the next pass
                        for msi in range(2):
                            ms = mp * 2 + msi
                            stats = p3.tile([P, E // 512, 6], f32, tag="ln_stats")
                            for i in range(E // 512):
                                nc.vector.bn_stats(
                                    out=stats[:, i, :],
                                    in_=xres[ms][:, i * 512:(i + 1) * 512])
                            nc.vector.bn_aggr(out=ln2_mv[ms], in_=stats)
                            nc.scalar.activation(
                                out=ln2_rstd[ms], in_=ln2_mv[ms][:, 1:2],
                                func=mybir.ActivationFunctionType.Sqrt, bias=eps_t)
                            nc.vector.reciprocal(out=ln2_rstd[ms], in_=ln2_rstd[ms])

        # ---------------- Phase 4: LN2 -> x2T strips; xres += bd ------------
        with tc.tile_pool(name="mlp_keep", bufs=1) as mlp_keep:
            x2T = mlp_keep.tile([P, NE, OWN], bf16, name="x2T")
            hT = [mlp_keep.tile([P, OWN], bf16, tag=f"hT{i}", name=f"hT{i}")
                  for i in range(NF)]
            wu_pre = [mlp_keep.tile([P, NE, P], bf16, tag=f"wupre{i}",
                                    name=f"wupre{i}") for i in range(2)]
            bd_sb = mlp_keep.tile([P, E], f32, name="bd_sb")
            nc.sync.dma_start(out=bd_sb, in_=bd_bc_d)

            with (
                tc.tile_pool(name="p4", bufs=1) as p4,
                tc.tile_pool(name="ps4", bufs=1, space="PSUM") as ps4,
            ):
                for i in range(2):
                    nc.sync.dma_start(out=wu_pre[i], in_=wu_s[i])
                for t in range(NMS):
                    x2_t = p4.tile([P, E], bf16, tag=f"x2_{t}", name=f"x2_{t}")
                    nc.vector.tensor_scalar(
                        out=x2_t, in0=xres[t], scalar1=ln2_mv[t][:, 0:1],
                        scalar2=ln2_rstd[t], op0=mybir.AluOpType.subtract,
                        op1=mybir.AluOpType.mult)
                    if STRIP_TP == 'dma':
                        for e in range(NE):
                            nc.sync.dma_start(
                                out=x2T[:, e, t * P:(t + 1) * P],
                                in_=x2_t[:, e * P:(e + 1) * P], transpose=True)
                    elif STRIP_TP == 'dram':
                        nc.sync.dma_start(
                            out=x1_scr[0, t * P:(t + 1) * P, :], in_=x2_t)
                    else:
                        for eg in range(4):
                            tp4 = ps4.tile([P, 4, P], f32, tag="tp4", bufs=2)
                            for i in range(4):
                                e = eg * 4 + i
                                nc.tensor.matmul(tp4[:, i, :],
                                                 x2_t[:, e * P:(e + 1) * P],
                                                 ident, start=True, stop=True)
                            for i in range(4):
                                e = eg * 4 + i
                                nc.scalar.copy(x2T[:, e, t * P:(t + 1) * P],
                                               tp4[:, i, :])
                    # bd folded into the residual stream after LN2 stats taken
                    nc.vector.tensor_tensor(out=xres[t], in0=xres[t],
                                            in1=bd_sb, op=add)
                if STRIP_TP == 'dram':
                    for e in range(NE):
                        nc.sync.dma_start(
                            out=x2T[:, e, :],
                            in_=x1_scr[0, :, e * P:(e + 1) * P], transpose=True)

            # ---------------- Phase 5: MLP up (gelu) -> hT ------------------
            with (
                tc.tile_pool(name="p5", bufs=1) as p5,
                tc.tile_pool(name="ps5", bufs=1, space="PSUM") as ps5,
            ):
                bu_sb = p5.tile([P, NF], f32)
                nc.sync.dma_start(out=bu_sb, in_=bu)
                for f in range(NF):
                    if f < 2:
                        wstrip = wu_pre[f]
                    else:
                        wstrip = p5.tile([P, NE, P], bf16, tag=f"wu{f % 2}", bufs=3)
                        nc.sync.dma_start(out=wstrip, in_=wu_s[f])
                    psh = ps5.tile([P, OWN], f32, tag=f"psh{f % 2}", bufs=2)
                    for e in range(NE):
                        nc.tensor.matmul(psh, wstrip[:, e, :], x2T[:, e, :],
                                         start=(e == 0), stop=(e == NE - 1))
                    nc.scalar.activation(out=hT[f], in_=psh, func=Gelu,
                                         bias=bu_sb[:, f:f + 1])

            # ------------- Phase 6: MLP down (natural) + residual -----------
            with (
                tc.tile_pool(name="p6", bufs=1) as p6,
                tc.tile_pool(name="ps6", bufs=1, space="PSUM") as ps6,
            ):
                for eg in range(4):
                    psd = [ps6.tile([P, OWN], f32, tag=f"psd{eg % 2}_{i}", bufs=1,
                                    name=f"psd{eg % 2}_{i}") for i in range(4)]
                    for fi in range(NF):
                        wtile = p6.tile([P, OWN], bf16, tag=f"wd{fi % 2}", bufs=4)
                        nc.sync.dma_start(
                            out=wtile, in_=wd_r[fi][:, eg * OWN:(eg + 1) * OWN])
                        for qi in range(4):
                            nc.tensor.matmul(psd[qi],
                                             hT[fi][:, qi * P:(qi + 1) * P],
                                             wtile, start=(fi == 0),
                                             stop=(fi == NF - 1))
                    for qi in range(4):
                        lo = eg * OWN
                        nc.vector.tensor_tensor(
                            out=xres[qi][:, lo:lo + OWN], in0=psd[qi],
                            in1=xres[qi][:, lo:lo + OWN], op=add)
                        nc.sync.dma_start(
                            out=out_d[qi * P:(qi + 1) * P, lo:lo + OWN],
                            in_=xres[qi][:, lo:lo + OWN])


_NC_CACHE = None
LAST_RESULTS = None


def _get_nc():
    global _NC_CACHE
    if _NC_CACHE is None:
        nc = build()
        split_waits(nc)
        _NC_CACHE = nc
    return _NC_CACHE


def _q8(w, out_shape_fn=None):
    """absmax-quantize to fp8e4 (clip 240); returns (q, dequant_scale)."""
    amax = float(np.abs(w).max())
    s = 240.0 * 0.98 / max(amax, 1e-30)
    q = np.clip(w * s, -240, 240).astype(ml_dtypes.float8_e4m3)
    return q, 1.0 / s


def _prep_shared(ln1_g, ln1_b, wq, bq, wk, bk, wv, bv, wo, bo, ln2_g, ln2_b,
                 wu, bu, wd, bd):
    f = np.float64
    ln1_g, ln1_b = np.asarray(ln1_g, f), np.asarray(ln1_b, f)
    ln2_g, ln2_b = np.asarray(ln2_g, f), np.asarray(ln2_b, f)
    wq, wk, wv = np.asarray(wq, f), np.asarray(wk, f), np.asarray(wv, f)
    wo, wu, wd = np.asarray(wo, f), np.asarray(wu, f), np.asarray(wd, f)
    # fold LN gains into weights, LN biases into projection biases
    wq_f, bq_f = ln1_g[:, None] * wq, np.asarray(bq, f) + ln1_b @ wq
    wk_f, bk_f = ln1_g[:, None] * wk, np.asarray(bk, f) + ln1_b @ wk
    wv_f, bv_f = ln1_g[:, None] * wv, np.asarray(bv, f) + ln1_b @ wv
    wu_f, bu_f = ln2_g[:, None] * wu, np.asarray(bu, f) + ln2_b @ wu
    # V bias folds through attention (softmax rows sum to 1) into bo;
    # each kv head's bias is shared by its G query heads (GQA)
    bv_full = np.repeat(bv_f.reshape(KH, D), G, axis=0).reshape(H * D)
    bo_f = np.asarray(bo, f) + bv_full @ wo

    def strips(w, n, dt):  # [E, n*128] -> [n, 128(p), NE, 128(m)]
        return np.ascontiguousarray(
            w.reshape(NE, P, n, P).transpose(2, 1, 0, 3)).astype(dt)

    def rows(w, nr):   # [nr*128, E] -> [nr, 128, E]
        return np.ascontiguousarray(w.reshape(nr, P, E)).astype(ml_dtypes.bfloat16)

    tri = np.triu(np.ones((OWN, OWN), np.float32))  # [key, query]: k <= q
    tri = np.ascontiguousarray(
        tri.reshape(NMS, P, OWN).transpose(1, 0, 2)).astype(ml_dtypes.bfloat16)

    def ptile(v, n):  # [n*128] -> [128, n] (partition-major)
        return np.ascontiguousarray(
            np.asarray(v).reshape(n, P).T).astype(np.float32)

    shared = {
        "wu_s": strips(wu_f, NF, ml_dtypes.bfloat16), "wd_r": rows(wd, NF),
        "bq": ptile(bq_f, H), "bk": ptile(bk_f, KH),
        "bu": ptile(bu_f, NF),
        "bd_bc": np.ascontiguousarray(
            np.broadcast_to(np.asarray(bd, f)[None, :], (P, E))).astype(np.float32),
        "tri": tri,
    }
    if FP8:
        wq8, dqq = _q8(wq_f)
        wk8, dqk = _q8(wk_f)
        wv8, dqv = _q8(wv_f)
        wo8, dqo = _q8(wo)
        shared["wq_s"] = strips(wq8.astype(f), H, ml_dtypes.float8_e4m3)
        shared["wk_s"] = strips(wk8.astype(f), KH, ml_dtypes.float8_e4m3)
        shared["wv_n"] = np.ascontiguousarray(
            wv8.reshape(NE, P, KH * D))
        if FP8_O:
            # wo pairs: [H//2, 128, 2, E]
            shared["wo_r"] = np.ascontiguousarray(
                wo8.reshape(H // 2, 2, P, E).transpose(0, 2, 1, 3))
        else:
            shared["wo_r"] = rows(wo, H)
            dqo = 1.0
        sv_eff = SV if FP8_AV else 1.0
        dqcols = np.array([dqq / SX, dqk / SX, dqv * sv_eff / SX,
                           dqo / sv_eff], np.float32)
        shared["dq"] = np.ascontiguousarray(
            np.broadcast_to(dqcols[None, :], (P, 4))).astype(np.float32)
    else:
        shared["wq_s"] = strips(wq_f, H, ml_dtypes.bfloat16)
        shared["wk_s"] = strips(wk_f, KH, ml_dtypes.bfloat16)
        shared["wv_n"] = np.ascontiguousarray(
            wv_f.reshape(NE, P, KH * D)).astype(ml_dtypes.bfloat16)
        shared["wo_r"] = rows(wo, H)
        shared["dq"] = np.ones((P, 4), np.float32)
    return shared, bo_f


def kernel(x, ln1_g, ln1_b, wq, bq, wk, bk, wv, bv, wo, bo, ln2_g, ln2_b,
           wu, bu, wd, bd):
    x = np.asarray(x, np.float32)
    shared, bo_f = _prep_shared(ln1_g, ln1_b, wq, bq, wk, bk, wv, bv, wo, bo,
                                ln2_g, ln2_b, wu, bu, wd, bd)
    in_maps = []
    for core in range(8):
        b, j = divmod(core, 4)
        m = dict(shared)
        own = slice(OWN * j, OWN * (j + 1))
        # reorder: own tokens first, then the rest in natural order
        order = np.concatenate([np.arange(OWN * j, OWN * (j + 1)),
                                np.arange(0, OWN * j),
                                np.arange(OWN * (j + 1), S)])
        m["xkv"] = np.ascontiguousarray(x[b][order]).astype(ml_dtypes.bfloat16)
        m["xres"] = np.ascontiguousarray(x[b, own] + bo_f[None, :]).astype(np.float32)
        # per-key additive bias: 0 if key visible to all own queries (or own),
        # NEGB if hidden from all own queries
        kb = np.where(order < OWN * (j + 1), -EXPC if FP8_AV else 0.0,
                      NEGB - (EXPC if FP8_AV else 0.0)).astype(np.float32)
        m["kbias"] = np.ascontiguousarray(kb.reshape(NSK, P).T).astype(np.float32)
        in_maps.append(m)

    nc = _get_nc()
    trace = bool(os.environ.get("KERNEL_TRACE"))
    res = bass_utils.run_bass_kernel_spmd(
        nc, in_maps, core_ids=list(range(8)), trace=trace)
    global LAST_RESULTS
    LAST_RESULTS = res
    out = np.empty((B, S, E), np.float32)
    for core in range(8):
        b, j = divmod(core, 4)
        out[b, OWN * j:OWN * (j + 1)] = res.results[core]["out"]
    return out


# revision 19
# speedup vs baseline: 1.4842x; 1.1843x over previous
"""GPT-2 transformer block on 8 trn2 NeuronCores (Bass/Tile), v4.

Sharding: token-split. Core c = 4*b + j handles batch b, output tokens
[512j, 512j+512). Host reorders each core's sequence so the own tokens sit at
positions [0,512); K/V are computed for the full (reordered) sequence, Q and
everything downstream only for positions [0,512). Causal masking:
  - non-own keys are visible to all own queries or none (per-key), applied as
    an additive bias (-30) inside the exp activation (per-partition bias);
  - own keys (score tiles 0..3) get a triangular mask multiply.
v3: DMA-XBAR transposes, token-major V (bias folded into bo), bit-trick
    Newton reciprocal + head-pipelined softmax tail, paired exp activations,
    gpsimd mask multiplies, natural-layout MLP down-projection.
v4 (KERNEL_FP8=1): Q/K/V projections, the att@V matmuls (non-diagonal pairs)
    and the o-projection run in fp8e4 with DoubleRow (2 contraction rows per
    pass); per-tensor absmax scales travel in the `dq` input. MLP stays bf16
    (fp8 there breaks the 2e-2 error budget; measured offline).
KERNEL_SCHR=1: exp for the last two key-tile pairs is computed on the vector
    engine via the Schraudolph bit trick (bf16 out), rebalancing the
    scalar-engine exp bottleneck.
"""
import math
import os
import sys
import types

sys.path.insert(0, '/opt/trn_rl_repo')

import numpy as np
import ml_dtypes


def _install_ntff_shim():
    """concourse's trace path imports antenv.axon_hooks, which this image
    lacks; give it a functional stand-in so trace=True doesn't crash."""
    try:
        import antenv.axon_hooks  # noqa: F401
        return
    except ImportError:
        pass
    try:
        import antenv
    except ImportError:
        return
    mod = types.ModuleType("antenv.axon_hooks")
    mod._hook = None

    def set_axon_ntff_profile_hook(h):
        mod._hook = h

    def get_axon_ntff_profile_hook():
        return mod._hook

    mod.set_axon_ntff_profile_hook = set_axon_ntff_profile_hook
    mod.get_axon_ntff_profile_hook = get_axon_ntff_profile_hook
    sys.modules["antenv.axon_hooks"] = mod
    antenv.axon_hooks = mod
    try:
        from trn_agent_boot.trn_boot import _ntff_profile_via_ctypes
        hook = _ntff_profile_via_ctypes('/opt/axon/libaxon_pjrt.so')
        if hook is not None:
            set_axon_ntff_profile_hook(hook)
    except Exception:
        pass


_install_ntff_shim()

import concourse.bass as bass
import concourse.tile as tile
from concourse import mybir, bass_utils
from concourse.masks import make_identity

P = 128
B, S, E = 2, 2048, 2048
H, D, KH, G = 16, 128, 4, 4
F = 8192
OWN = 512                 # tokens owned per core
NE = E // P               # 16
NSK = S // P              # 16
NF = F // P               # 64
NMS = OWN // P            # 4
f32 = mybir.dt.float32
f32r = mybir.dt.float32r
i32 = mybir.dt.int32
i16 = mybir.dt.int16
bf16 = mybir.dt.bfloat16
fp8 = mybir.dt.float8e4
EXP_SCALE = 1.0 / math.sqrt(D)
NEGB = -30.0              # additive key bias for hidden keys (exp->~1e-13)
EXPC = 3.5                # subtract from every logit: keeps exp < 240 so the
                          # fp8e4 store can't hit TRN's (240,448]->NaN band;
                          # cancels exactly in softmax (den shifts too)
RCP_MAGIC = 0x7EF311C3    # fast-reciprocal seed; 1 Newton step -> ~0.26% max err
# Schraudolph exp, bf16-bits variant: bits16 = x*SCHR_A + (A*kb + SCHR_B)
SCHR_A0 = 12102203.161561485          # 2^23 / ln 2
SCHR_B0 = 1064986823.0 - 0.014 * 8388608.0   # centered (mean ratio ~1)
SX = 8.0                  # fp8 scale for LN1 output (|x1| < 30 whp)
SV = 16.0                 # fp8 scale for V values
SO = 16.0                 # fp8 scale for attention output
STRIP_TP = os.environ.get("KERNEL_STRIP_TP", "mm")  # 'dma' (XBAR) or 'mm' (PE)
MASK_ENG = os.environ.get("KERNEL_MASK_ENG", "gpsimd")  # 'gpsimd' or 'vector'
FP8 = os.environ.get("KERNEL_FP8", "1") == "1"
FP8_AV = FP8 and os.environ.get("KERNEL_FP8_AV", "1") == "1"
FP8_O = FP8 and os.environ.get("KERNEL_FP8_O", "1") == "1"
SCHR = os.environ.get("KERNEL_SCHR", "0") == "1"
DR = mybir.MatmulPerfMode.DoubleRow


def split_waits(nc, maxw=1):
    """This walrus build supports at most one sync-wait per instruction;
    hoist excess waits onto same-engine NoOps placed before the owner."""
    n = 0
    for fn in nc.m.functions:
        for blk in fn.blocks:
            new_insts = []
            for inst in blk.instructions:
                si = inst.sync_info
                if si is not None and si.on_wait and len(si.on_wait) > maxw:
                    waits = list(si.on_wait)
                    excess, keep = waits[:-maxw], waits[-maxw:]
                    for ci, w in enumerate(excess):
                        new_insts.append(mybir.InstNoOp(
                            name=f"{inst.name}-ws{ci}", engine=inst.engine,
                            sync_info=mybir.SyncInfo(on_wait=[w], on_update=[])))
                        n += 1
                    inst.sync_info = mybir.SyncInfo(
                        on_wait=keep, on_update=list(si.on_update or []))
                new_insts.append(inst)
            blk.instructions = new_insts
    return n


def _ln_tile(nc, pool, x_tile, eps_t, out_tile, post_scale=None):
    """out = (x - mean(x)) * rsqrt(var(x) + eps) [* post_scale] along free dim."""
    stats = pool.tile([P, E // 512, 6], f32, tag="ln_stats")
    for i in range(E // 512):
        nc.vector.bn_stats(out=stats[:, i, :], in_=x_tile[:, i * 512:(i + 1) * 512])
    mv = pool.tile([P, 2], f32, tag="ln_mv")
    nc.vector.bn_aggr(out=mv, in_=stats)
    rstd = pool.tile([P, 1], f32, tag="ln_rstd")
    nc.scalar.activation(out=rstd, in_=mv[:, 1:2],
                         func=mybir.ActivationFunctionType.Sqrt, bias=eps_t)
    nc.vector.reciprocal(out=rstd, in_=rstd)
    if post_scale is not None:
        nc.vector.tensor_scalar(out=rstd, in0=rstd, scalar1=post_scale,
                                scalar2=None, op0=mybir.AluOpType.mult)
    nc.vector.tensor_scalar(out=out_tile, in0=x_tile, scalar1=mv[:, 0:1],
                            scalar2=rstd, op0=mybir.AluOpType.subtract,
                            op1=mybir.AluOpType.mult)


def build():
    nc = bass.Bass("TRN2", target_bir_lowering=False, debug=False, num_devices=8)

    wdt = fp8 if FP8 else bf16
    xkv = nc.dram_tensor("xkv", [S, E], bf16, kind="ExternalInput").ap()
    xres = nc.dram_tensor("xres", [OWN, E], f32, kind="ExternalInput").ap()
    tri = nc.dram_tensor("tri", [P, NMS, OWN], bf16, kind="ExternalInput").ap()
    kbias = nc.dram_tensor("kbias", [P, NSK], f32, kind="ExternalInput").ap()
    wq_s = nc.dram_tensor("wq_s", [H, P, NE, P], wdt, kind="ExternalInput").ap()
    wk_s = nc.dram_tensor("wk_s", [KH, P, NE, P], wdt, kind="ExternalInput").ap()
    wv_n = nc.dram_tensor("wv_n", [NE, P, KH * D], wdt, kind="ExternalInput").ap()
    if FP8 and os.environ.get("KERNEL_FP8_O", "1") == "1":
        wo_r = nc.dram_tensor("wo_r", [H // 2, P, 2, E], fp8,
                              kind="ExternalInput").ap()
    else:
        wo_r = nc.dram_tensor("wo_r", [H, P, E], bf16, kind="ExternalInput").ap()
    wu_s = nc.dram_tensor("wu_s", [NF, P, NE, P], bf16, kind="ExternalInput").ap()
    wd_r = nc.dram_tensor("wd_r", [NF, P, E], bf16, kind="ExternalInput").ap()
    bq = nc.dram_tensor("bq", [P, H], f32, kind="ExternalInput").ap()
    bk = nc.dram_tensor("bk", [P, KH], f32, kind="ExternalInput").ap()
    bu = nc.dram_tensor("bu", [P, NF], f32, kind="ExternalInput").ap()
    bd_bc = nc.dram_tensor("bd_bc", [P, E], f32, kind="ExternalInput").ap()
    dq = nc.dram_tensor("dq", [P, 4], f32, kind="ExternalInput").ap()
    x1_scr = nc.dram_tensor("x1_scr", [2, OWN, E], bf16, kind="Internal").ap()
    out_d = nc.dram_tensor("out", [OWN, E], f32, kind="ExternalOutput").ap()

    with tile.TileContext(nc) as tc:
        _build_body(nc, tc, locals())
    return nc


def _build_body(nc, tc, t_):
    xkv, xres_d, tri_d, kbias_d = t_["xkv"], t_["xres"], t_["tri"], t_["kbias"]
    x1_scr = t_["x1_scr"]
    wq_s, wk_s, wv_n, wo_r, wu_s, wd_r = (t_[k] for k in
                                          ("wq_s", "wk_s", "wv_n", "wo_r", "wu_s", "wd_r"))
    bq, bk, bu, bd_bc_d, dq_d = (t_[k] for k in ("bq", "bk", "bu", "bd_bc", "dq"))
    out_d = t_["out_d"]
    del t_
    Ident = mybir.ActivationFunctionType.Identity
    Exp = mybir.ActivationFunctionType.Exp
    Gelu = mybir.ActivationFunctionType.Gelu
    mult = mybir.AluOpType.mult
    add = mybir.AluOpType.add

    mask_eng = nc.gpsimd if MASK_ENG == 'gpsimd' else nc.vector

    with (
        tc.tile_pool(name="persist", bufs=1) as persist,
        tc.tile_pool(name="resid", bufs=1) as resid,
    ):
        eps_t = persist.tile([P, 1], f32)
        nc.vector.memset(eps_t, 1e-5)
        ones_col = persist.tile([P, 1], bf16)   # lhsT for denominator (K=P, M=1)
        nc.vector.memset(ones_col, 1.0)
        sv_eff = SV if FP8_AV else 1.0
        ones_row = persist.tile([1, P], f32r)   # lhsT for broadcast (K=1, M=P)
        nc.vector.memset(ones_row.bitcast(f32), 1.0 if FP8_O else 1.0 / sv_eff)
        # DR lhsT for fp8 denominator; ko-step must be 16B-aligned, so pad
        ones8_t = persist.tile([P, 2, 16], fp8)
        nc.vector.memset(ones8_t, 1.0)
        ones8 = ones8_t[:, :, 0:1]
        if STRIP_TP == 'mm':
            ident = persist.tile([P, P], bf16)
            make_identity(nc, ident)
        bq_sb = persist.tile([P, H], f32)
        nc.sync.dma_start(out=bq_sb, in_=bq)
        bk_sb = persist.tile([P, KH], f32)
        nc.sync.dma_start(out=bk_sb, in_=bk)
        kb_sb = persist.tile([P, NSK], f32)     # per-key additive exp bias
        nc.sync.dma_start(out=kb_sb, in_=kbias_d)
        dq_sb = persist.tile([P, 4], f32)       # fp8 dequant scales
        nc.sync.dma_start(out=dq_sb, in_=dq_d)
        if SCHR:  # kb2 = kb*(A/2^16) + B/2^16 for the bf16-bits exp trick
            kb2_sb = persist.tile([P, NSK], f32)
            nc.vector.tensor_scalar(
                out=kb2_sb, in0=kb_sb, scalar1=SCHR_A0 / 65536.0,
                scalar2=SCHR_B0 / 65536.0, op0=mult, op1=add)
        tri_sb = persist.tile([P, NMS, OWN], bf16)  # triangular mask tiles

        with tc.tile_pool(name="qkv_keep", bufs=1) as qkv_keep:
            qT = [qkv_keep.tile([P, OWN], bf16, tag=f"qT{i}", name=f"qT{i}") for i in range(H)]
            kT = [qkv_keep.tile([P, S], bf16, tag=f"kT{i}", name=f"kT{i}") for i in range(KH)]
            vdt = fp8 if FP8_AV else bf16
            vtok = qkv_keep.tile([P, NSK, KH * D], vdt, name="vtok")
            xres = [resid.tile([P, E], f32, tag=f"xres{t}", name=f"xres{t}")
                    for t in range(NMS)]
            ln2_mv = [resid.tile([P, 2], f32, tag=f"l2m{t}", name=f"l2m{t}")
                      for t in range(NMS)]
            ln2_rstd = [resid.tile([P, 1], f32, tag=f"l2r{t}", name=f"l2r{t}")
                        for t in range(NMS)]

            # ---------------- Phase 1: LN1 + Q/K/V over reordered seq --------
            with (
                tc.tile_pool(name="p1", bufs=1) as p1,
                tc.tile_pool(name="ps1", bufs=1, space="PSUM") as ps1,
            ):
                # x for chunk 0 first: one wide DMA per chunk, 4 token-tiles
                x4s = []
                for c in range(S // OWN):
                    x4 = p1.tile([P, NMS, E], bf16, tag="x4", bufs=2, name=f"x4_{c}")
                    nc.sync.dma_start(
                        out=x4,
                        in_=xkv[c * OWN:(c + 1) * OWN, :].rearrange(
                            "(t p) e -> p t e", p=P))
                    x4s.append(x4)
                # wv tiles stay resident (reused by every chunk)
                wv_sb = p1.tile([P, NE, KH * D], fp8 if FP8 else bf16, name="wv_sb")
                nc.sync.dma_start(out=wv_sb, in_=wv_n.rearrange("e p m -> p e m"))

                def proj_mms(psum, wstrip, strip, n=NE):
                    if FP8:
                        for e in range(0, n, 2):
                            nc.tensor.matmul(psum, wstrip[:, e:e + 2, :],
                                             strip[:, e:e + 2, :],
                                             start=(e == 0), stop=(e == n - 2),
                                             perf_mode=DR)
                    else:
                        for e in range(n):
                            nc.tensor.matmul(psum, wstrip[:, e, :], strip[:, e, :],
                                             start=(e == 0), stop=(e == n - 1))

                for c in range(S // OWN):
                    x4 = x4s[c] if c < 2 else p1.tile(
                        [P, NMS, E], bf16, tag="x4", bufs=2, name=f"x4_{c}")
                    if c >= 2:
                        nc.sync.dma_start(
                            out=x4,
                            in_=xkv[c * OWN:(c + 1) * OWN, :].rearrange(
                                "(t p) e -> p t e", p=P))
                    strip_b = p1.tile([P, NE, OWN], bf16, tag="strip", bufs=2,
                                      name=f"strip{c}")
                    x1c = []
                    for t in range(NMS):
                        x1_t = p1.tile([P, E], bf16, tag=f"x1_{t}")
                        _ln_tile(nc, p1, x4[:, t, :], eps_t, x1_t,
                                 post_scale=SX if FP8 else None)
                        x1c.append(x1_t)
                        if STRIP_TP == 'dma':
                            for e in range(NE):
                                nc.sync.dma_start(
                                    out=strip_b[:, e, t * P:(t + 1) * P],
                                    in_=x1_t[:, e * P:(e + 1) * P],
                                    transpose=True)
                        elif STRIP_TP == 'dram':
                            nc.sync.dma_start(
                                out=x1_scr[c % 2, t * P:(t + 1) * P, :],
                                in_=x1_t)
                    if STRIP_TP == 'dram':
                        for e in range(NE):
                            nc.sync.dma_start(
                                out=strip_b[:, e, :],
                                in_=x1_scr[c % 2, :, e * P:(e + 1) * P],
                                transpose=True)
                    if STRIP_TP == 'mm':
                        for e in range(NE):
                            tp4 = ps1.tile([P, NMS, P], f32, tag="tp4", bufs=2)
                            for t in range(NMS):
                                nc.tensor.matmul(tp4[:, t, :],
                                                 x1c[t][:, e * P:(e + 1) * P],
                                                 ident, start=True, stop=True)
                            nc.scalar.copy(strip_b[:, e, :],
                                           tp4.rearrange("p t q -> p (t q)"))
                    if FP8:
                        strip = p1.tile([P, NE, OWN], fp8, tag="strip8", bufs=2,
                                        name=f"strip8_{c}")
                        nc.vector.tensor_copy(
                            strip.rearrange("p e q -> p (e q)"),
                            strip_b.rearrange("p e q -> p (e q)"))
                    else:
                        strip = strip_b
                    # K projection for this chunk (d-major, like v2)
                    for m in range(KH):
                        wstrip = p1.tile([P, NE, P], wq_s.dtype, tag=f"w{m % 2}",
                                         bufs=2)
                        nc.sync.dma_start(out=wstrip, in_=wk_s[m])
                        pskv = ps1.tile([P, OWN], f32, tag=f"ps{m % 2}", bufs=2)
                        proj_mms(pskv, wstrip, strip)
                        nc.scalar.activation(
                            out=kT[m][:, c * OWN:(c + 1) * OWN],
                            in_=pskv, func=Ident, bias=bk_sb[:, m:m + 1],
                            scale=dq_sb[:, 1:2] if FP8 else 1.0)
                    # V projection, directly token-major (bias folded into bo)
                    for t in range(NMS):
                        psv = ps1.tile([P, KH * D], f32, tag=f"ps{t % 2}", bufs=2)
                        if FP8:
                            for e in range(0, NE, 2):
                                nc.tensor.matmul(
                                    psv, strip[:, e:e + 2, t * P:(t + 1) * P],
                                    wv_sb[:, e:e + 2, :],
                                    start=(e == 0), stop=(e == NE - 2),
                                    perf_mode=DR)
                            nc.scalar.activation(out=vtok[:, c * NMS + t, :],
                                                 in_=psv, func=Ident,
                                                 scale=dq_sb[:, 2:3])  # = SV/(SX*swv)
                        else:
                            for e in range(NE):
                                nc.tensor.matmul(
                                    psv, strip[:, e, t * P:(t + 1) * P],
                                    wv_sb[:, e, :],
                                    start=(e == 0), stop=(e == NE - 1))
                            nc.scalar.copy(vtok[:, c * NMS + t, :], psv)
                    if c == 0:
                        # Q projections for own tokens (positions [0, 512))
                        for m in range(H):
                            wstrip = p1.tile([P, NE, P], wq_s.dtype,
                                             tag=f"w{m % 2}", bufs=2)
                            nc.sync.dma_start(out=wstrip, in_=wq_s[m])
                            psq = ps1.tile([P, OWN], f32, tag=f"ps{m % 2}", bufs=2)
                            proj_mms(psq, wstrip, strip)
                            nc.scalar.activation(
                                out=qT[m], in_=psq, func=Ident,
                                bias=bq_sb[:, m:m + 1],
                                scale=dq_sb[:, 0:1] if FP8 else 1.0)

            # ---------------- Phase 2: attention -> oT -----------------------
            with tc.tile_pool(name="oT_keep", bufs=1) as oT_keep:
                odt = fp8 if FP8_O else bf16
                oT = oT_keep.tile([P, H, OWN], odt, name="oT")
                if FP8_O:
                    wo_pre = [oT_keep.tile([P, 2, E], fp8, tag=f"wopre{i}",
                                           name=f"wopre{i}") for i in range(2)]
                else:
                    wo_pre = [oT_keep.tile([P, E], bf16, tag=f"wopre{i}",
                                           name=f"wopre{i}") for i in range(2)]
                with (
                    tc.tile_pool(name="p2", bufs=1) as p2,
                    tc.tile_pool(name="ps2", bufs=1, space="PSUM") as ps2,
                ):
                    nc.sync.dma_start(out=tri_sb, in_=tri_d)
                    for i in range(2):
                        nc.sync.dma_start(out=wo_pre[i], in_=wo_r[i])
                    for t in range(NMS):
                        nc.sync.dma_start(out=xres[t], in_=xres_d[t * P:(t + 1) * P, :])

                    LOOK = 2
                    NPAIR = NSK // 2
                    prev = None   # deferred softmax tail state of head h-1

                    def nr_recip(st):
                        with nc.allow_low_precision(reason="softmax recip"):
                            nc.vector.reciprocal(out=st["rden"], in_=st["ps_den"])

                    def bc_mm(st):
                        ps_bc = ps2.tile([P, OWN], f32, tag="ps_bc", bufs=1)
                        nc.tensor.matmul(ps_bc, ones_row, st["rden"],
                                         start=True, stop=True)
                        st["ps_bc"] = ps_bc

                    def final_mult(st):
                        # DVE cannot read two PSUM operands; stage bc in SBUF
                        bc = p2.tile([P, OWN], f32, tag="bc", bufs=2)
                        nc.vector.tensor_copy(bc, st["ps_bc"])
                        nc.vector.tensor_tensor(out=oT[:, st["h"], :],
                                                in0=st["ps_o"],
                                                in1=bc, op=mult)

                    for h in range(H):
                        kv = h // G
                        ps_o = ps2.tile([P, OWN], f32, tag="ps_o", bufs=2)
                        acc = p2.tile([P, 2, OWN], bf16, tag="acc", bufs=2)
                        exrs = {}

                        def issue_pair(pr, kv=kv, h=h, exrs=exrs):
                            # pairs 0,1 carry the triangular mask (bf16 path);
                            # SCHR pairs 6,7 use the vector-engine exp trick
                            schr_pr = SCHR and pr >= NPAIR - 2
                            plain8 = FP8_AV and not schr_pr and pr >= 2
                            ps_s = ps2.tile([P, 2, OWN], f32, tag="ps_s", bufs=2)
                            exr2 = p2.tile([P, 2, OWN], fp8 if plain8 else bf16,
                                           tag="exr8" if plain8 else "exr",
                                           bufs=3)
                            for i in (0, 1):
                                sk = 2 * pr + i
                                nc.tensor.matmul(
                                    ps_s[:, i, :], kT[kv][:, sk * P:(sk + 1) * P],
                                    qT[h], start=True, stop=True)
                            if schr_pr:
                                nc.vector.tensor_scalar(
                                    out=exr2.rearrange("p t q -> p (t q)"
                                                       ).bitcast(i16),
                                    in0=ps_s.rearrange("p t q -> p (t q)"),
                                    scalar1=SCHR_A0 * EXP_SCALE / 65536.0,
                                    scalar2=kb2_sb[:, 2 * pr:2 * pr + 1],
                                    op0=mult, op1=add)
                            else:
                                nc.scalar.activation(
                                    out=exr2.rearrange("p t q -> p (t q)"),
                                    in_=ps_s.rearrange("p t q -> p (t q)"),
                                    func=Exp, scale=EXP_SCALE,
                                    bias=kb_sb[:, 2 * pr:2 * pr + 1])
                            if pr < 2:  # diagonal: triangular mask multiply
                                mask_eng.tensor_tensor(
                                    out=exr2.rearrange("p t q -> p (t q)"),
                                    in0=exr2.rearrange("p t q -> p (t q)"),
                                    in1=tri_sb[:, 2 * pr:2 * pr + 2, :].rearrange(
                                        "p t q -> p (t q)"), op=mult)
                            exrs[pr] = (exr2, plain8)

                        ps_den = ps2.tile([1, OWN], f32, tag="ps_den", bufs=1)
                        nbf = 0  # bf16 pairs seen (their den goes via acc)
                        nf8 = 0
                        den_started = [False]

                        for pr in range(LOOK):
                            issue_pair(pr)
                        if prev is not None:
                            nr_recip(prev)
                        for pr in range(NPAIR):
                            if pr + LOOK < NPAIR:
                                issue_pair(pr + LOOK)
                            exr2, plain8 = exrs[pr]
                            if plain8:
                                nc.tensor.matmul(
                                    ps_o,
                                    vtok[:, 2 * pr:2 * pr + 2,
                                         kv * D:(kv + 1) * D],
                                    exr2, start=(pr == 0), stop=(pr == NPAIR - 1),
                                    perf_mode=DR)
                                # denominator ride-along on PE (DR, M=1)
                                nc.tensor.matmul(
                                    ps_den, ones8, exr2,
                                    start=not den_started[0], stop=False,
                                    perf_mode=DR, skip_group_check=True)
                                den_started[0] = True
                            else:
                                for i in (0, 1):
                                    sk = 2 * pr + i
                                    nc.tensor.matmul(
                                        ps_o, vtok[:, sk, kv * D:(kv + 1) * D],
                                        exr2[:, i, :], start=(sk == 0),
                                        stop=(sk == NSK - 1))
                                with nc.allow_low_precision(reason="softmax den"):
                                    if nbf == 0:
                                        nc.vector.tensor_copy(
                                            acc.rearrange("p t q -> p (t q)"),
                                            exr2.rearrange("p t q -> p (t q)"))
                                    else:
                                        nc.vector.tensor_tensor(
                                            out=acc.rearrange("p t q -> p (t q)"),
                                            in0=acc.rearrange("p t q -> p (t q)"),
                                            in1=exr2.rearrange("p t q -> p (t q)"),
                                            op=add)
                                nbf += 1
                            if pr == 2 and prev is not None:
                                bc_mm(prev)
                            if pr == 4 and prev is not None:
                                final_mult(prev)
                        nc.tensor.matmul(ps_den, ones_col, acc[:, 0, :],
                                         start=not den_started[0], stop=False,
                                         skip_group_check=True)
                        nc.tensor.matmul(ps_den, ones_col, acc[:, 1, :],
                                         start=False, stop=True,
                                         skip_group_check=True)
                        rden = p2.tile([1, OWN], f32r, tag="rden", bufs=2)
                        prev = {"h": h, "ps_o": ps_o, "ps_den": ps_den, "rden": rden}
                    # flush the last head's tail
                    nr_recip(prev)
                    bc_mm(prev)
                    final_mult(prev)

                # ---------------- Phase 3: o-proj + residual -> xres ---------
                with (
                    tc.tile_pool(name="p3", bufs=1) as p3,
                    tc.tile_pool(name="ps3", bufs=1, space="PSUM") as ps3,
                ):
                    for mp in range(2):
                        pso = [ps3.tile([P, OWN], f32, tag=f"pso{i}", bufs=1,
                                        name=f"pso{i}") for i in range(8)]
                        nk = H // 2 if FP8_O else H
                        for k in range(nk):
                            if k < 2:
                                wtile = wo_pre[k]
                            else:
                                wtile = p3.tile(
                                    [P, 2, E] if FP8_O else [P, E],
                                    fp8 if FP8_O else bf16, tag="wo", bufs=3)
                                nc.sync.dma_start(out=wtile, in_=wo_r[k])
                            for ec in range(4):
                                for msi in range(2):
                                    ms = mp * 2 + msi
                                    if FP8_O:
                                        nc.tensor.matmul(
                                            pso[msi * 4 + ec],
                                            oT[:, 2 * k:2 * k + 2,
                                               ms * P:(ms + 1) * P],
                                            wtile[:, :, ec * OWN:(ec + 1) * OWN],
                                            start=(k == 0), stop=(k == nk - 1),
                                            perf_mode=DR)
                                    else:
                                        nc.tensor.matmul(
                                            pso[msi * 4 + ec],
                                            oT[:, k, ms * P:(ms + 1) * P],
                                            wtile[:, ec * OWN:(ec + 1) * OWN],
                                            start=(k == 0), stop=(k == nk - 1))
                        for msi in range(2):
                            ms = mp * 2 + msi
                            for ec in range(4):
                                lo = ec * OWN
                                src = pso[msi * 4 + ec]
                                if FP8_O:  # dequant on the (idle) scalar engine
                                    t8 = p3.tile([P, OWN], f32, tag="t8", bufs=3)
                                    nc.scalar.activation(
                                        out=t8, in_=src, func=Ident,
                                        scale=dq_sb[:, 3:4])
                                    src = t8
                                nc.vector.tensor_tensor(
                                    out=xres[ms][:, lo:lo + OWN],
                                    in0=src,
                                    in1=xres[ms][:, lo:lo + OWN], op=add)
                        # LN2 stats for this pair overlap the next pass
                        for msi in range(2):
                            ms = mp * 2 + msi
                            stats = p3.tile([P, E // 512, 6], f32, tag="ln_stats")
                            for i in range(E // 512):
                                nc.vector.bn_stats(
                                    out=stats[:, i, :],
                                    in_=xres[ms][:, i * 512:(i + 1) * 512])
                            nc.vector.bn_aggr(out=ln2_mv[ms], in_=stats)
                            nc.scalar.activation(
                                out=ln2_rstd[ms], in_=ln2_mv[ms][:, 1:2],
                                func=mybir.ActivationFunctionType.Sqrt, bias=eps_t)
                            nc.vector.reciprocal(out=ln2_rstd[ms], in_=ln2_rstd[ms])

        # ---------------- Phase 4: LN2 -> x2T strips; xres += bd ------------
        with tc.tile_pool(name="mlp_keep", bufs=1) as mlp_keep:
            x2T = mlp_keep.tile([P, NE, OWN], bf16, name="x2T")
            hT = [mlp_keep.tile([P, OWN], bf16, tag=f"hT{i}", name=f"hT{i}")
                  for i in range(NF)]
            wu_pre = [mlp_keep.tile([P, NE, P], bf16, tag=f"wupre{i}",
                                    name=f"wupre{i}") for i in range(2)]
            bd_sb = mlp_keep.tile([P, E], f32, name="bd_sb")
            nc.sync.dma_start(out=bd_sb, in_=bd_bc_d)

            with (
                tc.tile_pool(name="p4", bufs=1) as p4,
                tc.tile_pool(name="ps4", bufs=1, space="PSUM") as ps4,
            ):
                for i in range(2):
                    nc.sync.dma_start(out=wu_pre[i], in_=wu_s[i])
                for t in range(NMS):
                    x2_t = p4.tile([P, E], bf16, tag=f"x2_{t}", name=f"x2_{t}")
                    nc.vector.tensor_scalar(
                        out=x2_t, in0=xres[t], scalar1=ln2_mv[t][:, 0:1],
                        scalar2=ln2_rstd[t], op0=mybir.AluOpType.subtract,
                        op1=mybir.AluOpType.mult)
                    if STRIP_TP == 'dma':
                        for e in range(NE):
                            nc.sync.dma_start(
                                out=x2T[:, e, t * P:(t + 1) * P],
                                in_=x2_t[:, e * P:(e + 1) * P], transpose=True)
                    elif STRIP_TP == 'dram':
                        nc.sync.dma_start(
                            out=x1_scr[0, t * P:(t + 1) * P, :], in_=x2_t)
                    else:
                        for eg in range(4):
                            tp4 = ps4.tile([P, 4, P], f32, tag="tp4", bufs=2)
                            for i in range(4):
                                e = eg * 4 + i
                                nc.tensor.matmul(tp4[:, i, :],
                                                 x2_t[:, e * P:(e + 1) * P],
                                                 ident, start=True, stop=True)
                            for i in range(4):
                                e = eg * 4 + i
                                nc.scalar.copy(x2T[:, e, t * P:(t + 1) * P],
                                               tp4[:, i, :])
                    # bd folded into the residual stream after LN2 stats taken
                    nc.vector.tensor_tensor(out=xres[t], in0=xres[t],
                                            in1=bd_sb, op=add)
                if STRIP_TP == 'dram':
                    for e in range(NE):
                        nc.sync.dma_start(
                            out=x2T[:, e, :],
                            in_=x1_scr[0, :, e * P:(e + 1) * P], transpose=True)

            # ---------------- Phase 5: MLP up (gelu) -> hT ------------------
            with (
                tc.tile_pool(name="p5", bufs=1) as p5,
                tc.tile_pool(name="ps5", bufs=1, space="PSUM") as ps5,
            ):
                bu_sb = p5.tile([P, NF], f32)
                nc.sync.dma_start(out=bu_sb, in_=bu)
                for f in range(NF):
                    if f < 2:
                        wstrip = wu_pre[f]
                    else:
                        wstrip = p5.tile([P, NE, P], bf16, tag=f"wu{f % 2}", bufs=3)
                        nc.sync.dma_start(out=wstrip, in_=wu_s[f])
                    psh = ps5.tile([P, OWN], f32, tag=f"psh{f % 2}", bufs=2)
                    for e in range(NE):
                        nc.tensor.matmul(psh, wstrip[:, e, :], x2T[:, e, :],
                                         start=(e == 0), stop=(e == NE - 1))
                    nc.scalar.activation(out=hT[f], in_=psh, func=Gelu,
                                         bias=bu_sb[:, f:f + 1])

            # ------------- Phase 6: MLP down (natural) + residual -----------
            with (
                tc.tile_pool(name="p6", bufs=1) as p6,
                tc.tile_pool(name="ps6", bufs=1, space="PSUM") as ps6,
            ):
                for eg in range(4):
                    psd = [ps6.tile([P, OWN], f32, tag=f"psd{eg % 2}_{i}", bufs=1,
                                    name=f"psd{eg % 2}_{i}") for i in range(4)]
                    for fi in range(NF):
                        wtile = p6.tile([P, OWN], bf16, tag=f"wd{fi % 2}", bufs=4)
                        nc.sync.dma_start(
                            out=wtile, in_=wd_r[fi][:, eg * OWN:(eg + 1) * OWN])
                        for qi in range(4):
                            nc.tensor.matmul(psd[qi],
                                             hT[fi][:, qi * P:(qi + 1) * P],
                                             wtile, start=(fi == 0),
                                             stop=(fi == NF - 1))
                    for qi in range(4):
                        lo = eg * OWN
                        nc.vector.tensor_tensor(
                            out=xres[qi][:, lo:lo + OWN], in0=psd[qi],
                            in1=xres[qi][:, lo:lo + OWN], op=add)
                        nc.sync.dma_start(
                            out=out_d[qi * P:(qi + 1) * P, lo:lo + OWN],
                            in_=xres[qi][:, lo:lo + OWN])


_NC_CACHE = None
LAST_RESULTS = None


def _get_nc():
    global _NC_CACHE
    if _NC_CACHE is None:
        nc = build()
        split_waits(nc)
        _NC_CACHE = nc
    return _NC_CACHE


def _q8(w, out_shape_fn=None):
    """absmax-quantize to fp8e4 (clip 240); returns (q, dequant_scale)."""
    amax = float(np.abs(w).max())
    s = 240.0 * 0.98 / max(amax, 1e-30)
    q = np.clip(w * s, -240, 240).astype(ml_dtypes.float8_e4m3)
    return q, 1.0 / s


def _prep_shared(ln1_g, ln1_b, wq, bq, wk, bk, wv, bv, wo, bo, ln2_g, ln2_b,
                 wu, bu, wd, bd):
    f = np.float64
    ln1_g, ln1_b = np.asarray(ln1_g, f), np.asarray(ln1_b, f)
    ln2_g, ln2_b = np.asarray(ln2_g, f), np.asarray(ln2_b, f)
    wq, wk, wv = np.asarray(wq, f), np.asarray(wk, f), np.asarray(wv, f)
    wo, wu, wd = np.asarray(wo, f), np.asarray(wu, f), np.asarray(wd, f)
    # fold LN gains into weights, LN biases into projection biases
    wq_f, bq_f = ln1_g[:, None] * wq, np.asarray(bq, f) + ln1_b @ wq
    wk_f, bk_f = ln1_g[:, None] * wk, np.asarray(bk, f) + ln1_b @ wk
    wv_f, bv_f = ln1_g[:, None] * wv, np.asarray(bv, f) + ln1_b @ wv
    wu_f, bu_f = ln2_g[:, None] * wu, np.asarray(bu, f) + ln2_b @ wu
    # V bias folds through attention (softmax rows sum to 1) into bo;
    # each kv head's bias is shared by its G query heads (GQA)
    bv_full = np.repeat(bv_f.reshape(KH, D), G, axis=0).reshape(H * D)
    bo_f = np.asarray(bo, f) + bv_full @ wo

    def strips(w, n, dt):  # [E, n*128] -> [n, 128(p), NE, 128(m)]
        return np.ascontiguousarray(
            w.reshape(NE, P, n, P).transpose(2, 1, 0, 3)).astype(dt)

    def rows(w, nr):   # [nr*128, E] -> [nr, 128, E]
        return np.ascontiguousarray(w.reshape(nr, P, E)).astype(ml_dtypes.bfloat16)

    tri = np.triu(np.ones((OWN, OWN), np.float32))  # [key, query]: k <= q
    tri = np.ascontiguousarray(
        tri.reshape(NMS, P, OWN).transpose(1, 0, 2)).astype(ml_dtypes.bfloat16)

    def ptile(v, n):  # [n*128] -> [128, n] (partition-major)
        return np.ascontiguousarray(
            np.asarray(v).reshape(n, P).T).astype(np.float32)

    shared = {
        "wu_s": strips(wu_f, NF, ml_dtypes.bfloat16), "wd_r": rows(wd, NF),
        "bq": ptile(bq_f, H), "bk": ptile(bk_f, KH),
        "bu": ptile(bu_f, NF),
        "bd_bc": np.ascontiguousarray(
            np.broadcast_to(np.asarray(bd, f)[None, :], (P, E))).astype(np.float32),
        "tri": tri,
    }
    if FP8:
        wq8, dqq = _q8(wq_f)
        wk8, dqk = _q8(wk_f)
        wv8, dqv = _q8(wv_f)
        wo8, dqo = _q8(wo)
        shared["wq_s"] = strips(wq8.astype(f), H, ml_dtypes.float8_e4m3)
        shared["wk_s"] = strips(wk8.astype(f), KH, ml_dtypes.float8_e4m3)
        shared["wv_n"] = np.ascontiguousarray(
            wv8.reshape(NE, P, KH * D))
        if FP8_O:
            # wo pairs: [H//2, 128, 2, E]
            shared["wo_r"] = np.ascontiguousarray(
                wo8.reshape(H // 2, 2, P, E).transpose(0, 2, 1, 3))
        else:
            shared["wo_r"] = rows(wo, H)
            dqo = 1.0
        sv_eff = SV if FP8_AV else 1.0
        dqcols = np.array([dqq / SX, dqk / SX, dqv * sv_eff / SX,
                           dqo / sv_eff], np.float32)
        shared["dq"] = np.ascontiguousarray(
            np.broadcast_to(dqcols[None, :], (P, 4))).astype(np.float32)
    else:
        shared["wq_s"] = strips(wq_f, H, ml_dtypes.bfloat16)
        shared["wk_s"] = strips(wk_f, KH, ml_dtypes.bfloat16)
        shared["wv_n"] = np.ascontiguousarray(
            wv_f.reshape(NE, P, KH * D)).astype(ml_dtypes.bfloat16)
        shared["wo_r"] = rows(wo, H)
        shared["dq"] = np.ones((P, 4), np.float32)
    return shared, bo_f


def kernel(x, ln1_g, ln1_b, wq, bq, wk, bk, wv, bv, wo, bo, ln2_g, ln2_b,
           wu, bu, wd, bd):
    x = np.asarray(x, np.float32)
    shared, bo_f = _prep_shared(ln1_g, ln1_b, wq, bq, wk, bk, wv, bv, wo, bo,
                                ln2_g, ln2_b, wu, bu, wd, bd)
    in_maps = []
    for core in range(8):
        b, j = divmod(core, 4)
        m = dict(shared)
        own = slice(OWN * j, OWN * (j + 1))
        # reorder: own tokens first, then the rest in natural order
        order = np.concatenate([np.arange(OWN * j, OWN * (j + 1)),
                                np.arange(0, OWN * j),
                                np.arange(OWN * (j + 1), S)])
        m["xkv"] = np.ascontiguousarray(x[b][order]).astype(ml_dtypes.bfloat16)
        m["xres"] = np.ascontiguousarray(x[b, own] + bo_f[None, :]).astype(np.float32)
        # per-key additive bias: 0 if key visible to all own queries (or own),
        # NEGB if hidden from all own queries
        kb = np.where(order < OWN * (j + 1), -EXPC if FP8_AV else 0.0,
                      NEGB - (EXPC if FP8_AV else 0.0)).astype(np.float32)
        m["kbias"] = np.ascontiguousarray(kb.reshape(NSK, P).T).astype(np.float32)
        in_maps.append(m)

    nc = _get_nc()
    trace = bool(os.environ.get("KERNEL_TRACE"))
    res = bass_utils.run_bass_kernel_spmd(
        nc, in_maps, core_ids=list(range(8)), trace=trace)
    global LAST_RESULTS
    LAST_RESULTS = res
    out = np.empty((B, S, E), np.float32)
    for core in range(8):
        b, j = divmod(core, 4)
        out[b, OWN * j:OWN * (j + 1)] = res.results[core]["out"]
    return out
